# revision 1
# baseline (speedup 1.0000x reference)
"""Trainium2 Bass kernel for the 2-layer heterogeneous GAT (drug/cell) model.

Strategy (8 NeuronCores, SPMD single program):
  - L1: all three relations (dd, dc, cc) partitioned by DST node block per
    core; edge-softmax segments stay local. Projections of the (replicated)
    input features are done on every core. No collectives in L1.
  - hd = o_dd exactly (drug semantic attention over M=1 is identity);
    layer-2 o_dd is dead code (final head reads cells only) and is skipped.
  - Softmax max-subtraction is dropped (values are O(1), exact in fp32):
    out[v] = (sum_e exp(e)·fs[src]) / (sum_e exp(e)).  Both segment sums are
    computed with one PE matmul per 128-edge chunk: lhsT = one-hot(dstloc),
    rhs = [w*fs | w], accumulated in PSUM over the chunks of each dst tile.
  - L2: dc edges partitioned by SRC drug block (gather tables stay local),
    giving per-core partial sums over all cells; AllToAll + local reduce
    delivers each core its own cell block.  cc edges partitioned by dst with
    a replicated table built from an AllGather of hc1^T.
  - Gathers use the batched int16 dma_gather (GPSIMD mlp library);
    drug tables (40960 rows) are split in two 20480-row halves per gather.
"""
import sys
sys.path.insert(0, '/opt/trn_rl_repo')
import os
import numpy as np

import concourse.bacc as bacc
import concourse.bass as bass
import concourse.tile as tile
from concourse import mybir, library_config

F32 = mybir.dt.float32
I16 = mybir.dt.int16
P = 128
H = 8
FD = 256          # feature dim
ROWW = 320        # gather-table row width (f32), 1280B, %256B
ERG = 256         # er/el group column offset inside table rows
AluOp = mybir.AluOpType
Act = mybir.ActivationFunctionType

FULL_CFG = dict(Nd=40000, Nc=10000, NDP=40960, NCP=10240,
                DBLK=5120, CBLK=1280, HALF=20480, n_cores=8)

N_CORES = 8


def legalize_waits(nc):
    """Split multi-wait instructions into wait-carrying NOP chains.

    The walrus build here accepts at most one sync-wait command per
    instruction (two for EventSemaphore); Tile attaches several.  Inserting
    NOPs immediately before the instruction on the same engine is
    semantically identical.
    """
    n_split = 0
    for fn in nc.m.functions:
        for bb in fn.blocks:
            insts = bb.instructions
            new = []
            changed = False
            for inst in insts:
                si = inst.sync_info
                waits = list(si.on_wait) if si is not None else []
                cap = 2 if isinstance(inst, mybir.InstEventSemaphore) else 1
                if len(waits) > cap:
                    keep = waits[-cap:]
                    for w in waits[:-cap]:
                        nop = mybir.InstNoOp(
                            name=nc.get_next_instruction_name(),
                            engine=inst.engine,
                            sync_info=mybir.SyncInfo(on_wait=[w], on_update=[]),
                            bass_nofuse=True,
                        )
                        new.append(nop)
                        n_split += 1
                    inst.sync_info = mybir.SyncInfo(
                        on_wait=keep, on_update=list(si.on_update))
                    changed = True
                new.append(inst)
            if changed:
                bb.instructions = new
    return n_split


# --------------------------------------------------------------------------
# host-side prep
# --------------------------------------------------------------------------

def _fold_weights(ip, cfg):
    """Fold attention vectors into projection matrices; build device weights."""
    def wel(W, a):  # W [256,256], a [H,D] -> [256,H]
        return (W.reshape(FD, H, 32) * a[None]).sum(-1)
    W = {}
    Wsrc, Wdst, al, ar = ip['Wsrc'], ip['Wdst_dc'], ip['attn_l'], ip['attn_r']
    # L1
    W['WDD'] = np.concatenate([Wsrc[0, 0], wel(Wsrc[0, 0], al[0, 0])], 1)   # [256,264]
    W['WerDD'] = wel(Wsrc[0, 0], ar[0, 0])                                   # [256,8]
    W['WDC'] = np.concatenate([Wsrc[0, 1], wel(Wsrc[0, 1], al[0, 1])], 1)
    W['WCC1'] = np.concatenate([Wsrc[0, 2], wel(Wsrc[0, 2], al[0, 2]),
                                wel(Wsrc[0, 2], ar[0, 2]),
                                wel(Wdst[0], ar[0, 1])], 1)                  # [256,280]
    # L2 (o_dd dead: only dc,cc)
    W['WDC2'] = np.concatenate([Wsrc[1, 1], wel(Wsrc[1, 1], al[1, 1])], 1)
    W['WCC2'] = np.concatenate([Wsrc[1, 2], wel(Wsrc[1, 2], al[1, 2]),
                                wel(Wsrc[1, 2], ar[1, 2]),
                                wel(Wdst[1], ar[1, 1])], 1)                  # [256,280]
    def padto(a, n):
        return np.concatenate([a, np.zeros((FD, n - a.shape[1]), a.dtype)], 1)
    out = {}
    out['WDD'] = padto(W['WDD'], ROWW).astype(np.float32).reshape(2, P, ROWW)
    out['WerDD'] = padto(W['WerDD'], 64).astype(np.float32).reshape(2, P, 64)
    out['WDC'] = padto(W['WDC'], ROWW).astype(np.float32).reshape(2, P, ROWW)
    out['WCC1'] = padto(W['WCC1'], ROWW).astype(np.float32).reshape(2, P, ROWW)
    out['WDC2'] = padto(W['WDC2'], ROWW).astype(np.float32).reshape(2, P, ROWW)
    out['WCC2'] = padto(W['WCC2'], ROWW).astype(np.float32).reshape(2, P, ROWW)
    out['bias5'] = np.stack([ip['gat_bias'][0, 0], ip['gat_bias'][0, 1],
                             ip['gat_bias'][0, 2], ip['gat_bias'][1, 1],
                             ip['gat_bias'][1, 2]]).astype(np.float32)       # [5,256]
    out['semW1'] = np.stack([ip['sem_W1'][0, 1].reshape(2, P, P),
                             ip['sem_W1'][1, 1].reshape(2, P, P)]).astype(np.float32)
    out['semb1'] = np.stack([ip['sem_b1'][0, 1].reshape(P, 1),
                             ip['sem_b1'][1, 1].reshape(P, 1)]).astype(np.float32)
    out['semW2'] = np.stack([ip['sem_W2'][0, 1].reshape(P, 1),
                             ip['sem_W2'][1, 1].reshape(P, 1)]).astype(np.float32)
    out['dnnW1'] = ip['dnn_W1'].reshape(2, P, 32).astype(np.float32)
    out['dnnb1'] = ip['dnn_b1'].reshape(32, 1).astype(np.float32)
    out['dnnW2'] = ip['dnn_W2'].astype(np.float32)                           # [32,16]
    out['dnnb2'] = ip['dnn_b2'].reshape(16, 1).astype(np.float32)
    out['dnnW3'] = ip['dnn_W3'].astype(np.float32)                           # [16,1]
    out['dnnb3'] = ip['dnn_b3'].reshape(1, 1).astype(np.float32)
    return out


def _wrap16(vals):
    """int16 values (len%128==0) -> wrapped [128, len/16] layout."""
    n = len(vals)
    out = np.zeros((P, n // 16), np.int16)
    a = np.asarray(vals, np.int16).reshape(-1, 16).T      # [16, n/16]
    for g in range(8):
        out[g * 16:(g + 1) * 16, :] = a
    return out


def _prep_rel(src, dst, cfg, *, part, n_tiles, halves, src_off=None,
              er_local_blk=None):
    """Build the per-core edge schedule for one relation instance.

    part: ('dst', blk) or ('src', blk) — which endpoint picks the core.
          For 'dst', tiles are local (dst - c*blk); for 'src', tiles are
          global over the dst space.
    halves: half size for the src gather table (None = single half).
    src_off: per-core offset subtracted from src (for core-local tables).
    er_local_blk: if set, er idx = dst - c*blk (core-local er table).
    Returns dict with nch (per tile per half, equalized over cores) and
    per-core streams (srcidx wrapped, eridx wrapped, dstloc f32).
    """
    nco = cfg['n_cores']
    kind, blk = part
    per_core = []
    for c in range(nco):
        if kind == 'dst':
            m = (dst >= c * blk) & (dst < (c + 1) * blk)
            dl = dst[m] - c * blk
        else:
            m = (src >= c * blk) & (src < (c + 1) * blk)
            dl = dst[m]
        s = src[m] - (c * src_off if src_off else 0)
        er = dst[m] - (c * er_local_blk if er_local_blk else 0)
        tile_id = dl // P
        per_core.append((s, dl % P, tile_id, er))
    nch = np.zeros((n_tiles, 2 if halves else 1), np.int64)
    buckets = []
    for c in range(nco):
        s, dloc, tid, erv = per_core[c]
        bk = {}
        for t in range(n_tiles):
            mt = tid == t
            st, dt_, ee = s[mt], dloc[mt], erv[mt]
            if halves:
                m0 = st < halves
                groups = [(st[m0], dt_[m0], ee[m0]),
                          (st[~m0] - halves, dt_[~m0], ee[~m0])]
            else:
                groups = [(st, dt_, ee)]
            bk[t] = groups
            for h, (gs, gd, gg) in enumerate(groups):
                nch[t, h] = max(nch[t, h], (len(gs) + P - 1) // P)
        buckets.append(bk)
    nch = np.maximum(nch, 1)  # at least one chunk so the psum group exists
    tot = int(nch.sum())
    srcs, ers, dls = [], [], []
    for c in range(nco):
        bk = buckets[c]
        sw = np.zeros((P, tot * 8), np.int16)
        ew = np.zeros((P, tot * 8), np.int16)
        dv = np.full((P, tot), -1.0, np.float32)
        col = 0
        for t in range(n_tiles):
            tile_er, tile_dl = [], []
            for h, (gs, gd, gg) in enumerate(bk[t]):
                n = int(nch[t, h]) * P
                a = np.zeros(n, np.int64)
                a[:len(gs)] = gs
                assert a.max(initial=0) < 32768
                sw[:, col * 8:(col + int(nch[t, h])) * 8] = _wrap16(a)
                tile_er.append((gg, n))
                dpad = np.full(n, -1.0, np.float32)
                dpad[:len(gd)] = gd
                tile_dl.append(dpad)
                col += int(nch[t, h])
            ern = np.zeros(sum(x[1] for x in tile_er), np.int64)
            off = 0
            for gg, n in tile_er:
                ern[off:off + len(gg)] = gg
                off += n
            assert ern.max(initial=0) < 32768
            tcol0 = col - int(nch[t].sum())
            ew[:, tcol0 * 8:col * 8] = _wrap16(ern)
            dall = np.concatenate(tile_dl)
            dv[:, tcol0:col] = dall.reshape(-1, P).T
        srcs.append(sw); ers.append(ew); dls.append(dv)
    return dict(nch=nch, src=srcs, er=ers, dl=dls, tot=tot)


def host_prep(ip, cfg):
    W = _fold_weights(ip, cfg)
    nco = cfg['n_cores']
    DBLK, CBLK, HALF = cfg['DBLK'], cfg['CBLK'], cfg['HALF']
    NDP, NCP = cfg['NDP'], cfg['NCP']
    DD_T, CT, DC2_T = DBLK // P, CBLK // P, NCP // P

    dd = _prep_rel(ip['src_dd'], ip['dst_dd'], cfg, part=('dst', DBLK),
                   n_tiles=DD_T, halves=HALF, er_local_blk=DBLK)
    dc1 = _prep_rel(ip['src_dc'], ip['dst_dc'], cfg, part=('dst', CBLK),
                    n_tiles=CT, halves=HALF)
    cc1 = _prep_rel(ip['src_cc'], ip['dst_cc'], cfg, part=('dst', CBLK),
                    n_tiles=CT, halves=None)
    dc2 = _prep_rel(ip['src_dc'], ip['dst_dc'], cfg, part=('src', DBLK),
                    n_tiles=DC2_T, halves=None, src_off=DBLK)
    cc2 = _prep_rel(ip['src_cc'], ip['dst_cc'], cfg, part=('dst', CBLK),
                    n_tiles=CT, halves=None)

    featD = np.zeros((NDP, FD), np.float32); featD[:cfg['Nd']] = ip['feat_drug']
    featC = np.zeros((NCP, FD), np.float32); featC[:cfg['Nc']] = ip['feat_cell']
    featDT = np.ascontiguousarray(featD.T).reshape(2, P, NDP)
    featCT = np.ascontiguousarray(featC.T).reshape(2, P, NCP)

    iota = np.broadcast_to(np.arange(P, dtype=np.float32), (P, P)).copy()
    ident = np.eye(P, dtype=np.float32)

    base = dict(featDT=featDT, featCT=featCT, iota=iota, ident=ident, **W)
    in_maps = []
    for c in range(nco):
        m = dict(base)
        m['featDTloc'] = np.ascontiguousarray(
            featD[c * DBLK:(c + 1) * DBLK].T).reshape(2, P, DBLK)
        m['dd_src'] = dd['src'][c]; m['dd_er'] = dd['er'][c]; m['dd_dl'] = dd['dl'][c]
        m['dc1_src'] = dc1['src'][c]; m['dc1_er'] = dc1['er'][c]; m['dc1_dl'] = dc1['dl'][c]
        m['cc1_src'] = cc1['src'][c]; m['cc1_er'] = cc1['er'][c]; m['cc1_dl'] = cc1['dl'][c]
        m['dc2_src'] = dc2['src'][c]; m['dc2_er'] = dc2['er'][c]; m['dc2_dl'] = dc2['dl'][c]
        m['cc2_src'] = cc2['src'][c]; m['cc2_er'] = cc2['er'][c]; m['cc2_dl'] = cc2['dl'][c]
        in_maps.append(m)
    sched = dict(dd=dd['nch'], dc1=dc1['nch'], cc1=cc1['nch'],
                 dc2=dc2['nch'], cc2=cc2['nch'])
    return sched, in_maps


# --------------------------------------------------------------------------
# device program
# --------------------------------------------------------------------------

def _proj_pass(nc, sb, ps, lhsT_dram, n_tiles, jobs, wtiles):
    """Project features into gather tables.

    lhsT_dram: [2, P, NCOLS] transposed features (DRAM input AP).
    jobs: list of (table_dram, ncols_payload, rhs_key) writing
          table rows [tile*P:(tile+1)*P, :ncols].
    wtiles: dict rhs_key -> (sbuf tile [P, ncols] chunk0, chunk1)
    """
    for t in range(n_tiles):
        lh0 = sb.tile([P, P], F32, tag="projlh")
        lh1 = sb.tile([P, P], F32, tag="projlh")
        nc.sync.dma_start(lh0[:], lhsT_dram[0, :, t * P:(t + 1) * P])
        nc.sync.dma_start(lh1[:], lhsT_dram[1, :, t * P:(t + 1) * P])
        for tab, ncols, wkey in jobs:
            w0, w1 = wtiles[wkey]
            pp = ps.tile([P, ncols], F32, space="PSUM", tag="projps")
            nc.tensor.matmul(pp[:], lhsT=lh0[:], rhs=w0[:], start=True, stop=False)
            nc.tensor.matmul(pp[:], lhsT=lh1[:], rhs=w1[:], start=False, stop=True)
            ot = sb.tile([P, ncols], F32, tag="projout")
            nc.scalar.copy(ot[:], pp[:])
            nc.sync.dma_start(tab[t * P:(t + 1) * P, :ncols], ot[:])


def _edge_phase(nc, sb, ps, consts, tab_views, er_view, streams, nch,
                epilogue, ncols_er_rel, er_step=ROWW):
    """One relation's edge phase.

    tab_views: list of per-half table APs (rows x ROWW) for the src gather.
    er_view:  AP [rows, 64] whose rel col range [ncols_er_rel:+8] holds er.
    streams:  (src16, er16, dl) DRAM APs.
    nch:      [T, n_halves] chunk counts.
    epilogue: fn(t, psum_ap, sb) consuming the accumulated [P,264] psum.
    """
    src16, er16, dlf = streams
    iota = consts['iota']
    T = nch.shape[0]
    col = 0
    for t in range(T):
        tile_nch = int(nch[t].sum())
        G = sb.tile([P, tile_nch, ROWW], F32, tag="G")
        E = sb.tile([P, tile_nch, 64], F32, tag="E")
        DL = sb.tile([P, tile_nch], F32, tag="DL")
        SI = sb.tile([P, tile_nch * 8], I16, tag="SI")
        EI = sb.tile([P, tile_nch * 8], I16, tag="EI")
        nc.sync.dma_start(DL[:], dlf[:, col:col + tile_nch])
        nc.sync.dma_start(SI[:], src16[:, col * 8:(col + tile_nch) * 8])
        nc.sync.dma_start(EI[:], er16[:, col * 8:(col + tile_nch) * 8])
        # src gathers (per half), split to <=256 idxs per call (SWDGE ring)
        GCH = 2
        ccol = 0
        for h in range(nch.shape[1]):
            nh = int(nch[t, h])
            for b0 in range(0, nh, GCH):
                nb = min(GCH, nh - b0)
                nidx = nb * P
                c0 = ccol + b0
                nc.gpsimd.dma_gather(
                    G[:, c0:c0 + nb, :], tab_views[h],
                    SI[:, c0 * 8:(c0 + nb) * 8], nidx, nidx, ROWW)
            ccol += nh
        # er gather, same split
        for b0 in range(0, tile_nch, GCH):
            nb = min(GCH, tile_nch - b0)
            nidx = nb * P
            nc.gpsimd.dma_gather(
                E[:, b0:b0 + nb, :], er_view, EI[:, b0 * 8:(b0 + nb) * 8],
                nidx, nidx, 64, elem_step=er_step)
        # edge math, batched over the tile's chunks
        ww = sb.tile([P, tile_nch, 8], F32, tag="ww")
        nc.vector.tensor_tensor(
            out=ww[:], in0=G[:, :, ERG:ERG + 8],
            in1=E[:, :, ncols_er_rel:ncols_er_rel + 8], op=AluOp.add)
        nc.vector.scalar_tensor_tensor(
            out=ww[:], in0=ww[:], scalar=0.2, in1=ww[:],
            op0=AluOp.mult, op1=AluOp.max)
        nc.scalar.activation(ww[:], ww[:], Act.Exp)
        pp = ps.tile([P, 264], F32, space="PSUM", tag="edgeps")
        for k in range(tile_nch):
            S = sb.tile([P, P], F32, tag="S")
            nc.vector.tensor_scalar(S[:], iota[:], DL[:, k:k + 1], None,
                                    op0=AluOp.is_equal)
            rhs = sb.tile([P, 264], F32, tag="rhs")
            nc.vector.tensor_tensor(
                out=rhs[:, :FD].rearrange("p (h d) -> p h d", h=H),
                in0=G[:, k, :FD].rearrange("p (h d) -> p h d", h=H),
                in1=ww[:, k, :, None].to_broadcast([P, H, 32]),
                op=AluOp.mult)
            nc.vector.tensor_copy(rhs[:, FD:FD + 8], ww[:, k, :])
            nc.tensor.matmul(pp[:], lhsT=S[:], rhs=rhs[:],
                             start=(k == 0), stop=(k == tile_nch - 1))
        epilogue(t, pp, sb)
        col += tile_nch


def _normalize_elu(nc, sb, pp, bias_tile, out_tile):
    """out = elu(U/den + bias) from psum [P,264] -> out_tile [P,256] SBUF."""
    den = sb.tile([P, 8], F32, tag="den")
    nc.vector.tensor_scalar_max(den[:], pp[:, FD:FD + 8], 1e-30)
    rec = sb.tile([P, 8], F32, tag="rec")
    nc.vector.reciprocal(rec[:], den[:])
    x = sb.tile([P, FD], F32, tag="xnrm")
    nc.vector.tensor_tensor(
        out=x[:].rearrange("p (h d) -> p h d", h=H),
        in0=pp[:, :FD].rearrange("p (h d) -> p h d", h=H),
        in1=rec[:, :, None].to_broadcast([P, H, 32]), op=AluOp.mult)
    nc.vector.tensor_add(x[:], x[:], bias_tile[:])
    # elu(x) = relu(x) - relu(1 - exp(x))
    ex = sb.tile([P, FD], F32, tag="eluex")
    nc.scalar.activation(ex[:], x[:], Act.Exp)
    nc.scalar.activation(ex[:], ex[:], Act.Relu, bias=1.0, scale=-1.0)
    nc.scalar.activation(out_tile[:], x[:], Act.Relu)
    nc.vector.tensor_sub(out_tile[:], out_tile[:], ex[:])


def _transpose_store(nc, sb, ps, consts, src_tile, dramT, col0):
    """src_tile [P,256] -> dramT[0:256, col0:col0+128] via two PE transposes."""
    for kk in range(2):
        tp = ps.tile([P, P], F32, space="PSUM", tag="tpps")
        nc.tensor.transpose(tp[:], src_tile[:, kk * P:(kk + 1) * P],
                            consts['ident'][:])
        ts = sb.tile([P, P], F32, tag="tpsb")
        nc.scalar.copy(ts[:], tp[:])
        nc.sync.dma_start(dramT[kk * P:(kk + 1) * P, col0:col0 + P], ts[:])


def _sem_combine(nc, sb, ps, consts, oDCT, oCCT, l, t, hc_cb):
    """Semantic attention over [o_dc, o_cc] for cell tile t (transposed).

    oDCT/oCCT: DRAM [256, CBLK]; result hcT halves passed to hc_cb(kk, tile).
    """
    W1 = consts['semW1'][l]     # 2 sbuf tiles [128,128]
    b1 = consts['semb1'][l]
    w2 = consts['semW2'][l]
    zT = []
    for src in (oDCT, oCCT):
        z0 = sb.tile([P, P], F32, tag="semz")
        z1 = sb.tile([P, P], F32, tag="semz")
        nc.sync.dma_start(z0[:], src[0:P, t * P:(t + 1) * P])
        nc.sync.dma_start(z1[:], src[P:2 * P, t * P:(t + 1) * P])
        zT.append((z0, z1))
    wms = []
    for m in range(2):
        hp = ps.tile([P, P], F32, space="PSUM", tag="aux")
        nc.tensor.matmul(hp[:], lhsT=W1[0][:], rhs=zT[m][0][:], start=True, stop=False)
        nc.tensor.matmul(hp[:], lhsT=W1[1][:], rhs=zT[m][1][:], start=False, stop=True)
        ht = sb.tile([P, P], F32, tag="semh")
        nc.scalar.activation(ht[:], hp[:], Act.Tanh, bias=b1[:])
        wp = ps.tile([1, P], F32, space="PSUM", tag="aux")
        nc.tensor.matmul(wp[:], lhsT=w2[:], rhs=ht[:], start=True, stop=True)
        wm = sb.tile([1, P], F32, tag="semw")
        nc.scalar.copy(wm[:], wp[:])
        wms.append(wm)
    beta = sb.tile([1, P], F32, tag="semb")
    nc.vector.tensor_sub(beta[:], wms[0][:], wms[1][:])
    nc.scalar.activation(beta[:], beta[:], Act.Sigmoid)
    bb = ps.tile([P, P], F32, space="PSUM", tag="aux")
    nc.tensor.matmul(bb[:], lhsT=consts['ones1'][:], rhs=beta[:], start=True, stop=True)
    for kk in range(2):
        diff = sb.tile([P, P], F32, tag="semd")
        nc.vector.tensor_sub(diff[:], zT[0][kk][:], zT[1][kk][:])
        nc.vector.tensor_mul(diff[:], diff[:], bb[:])
        hct = sb.tile([P, P], F32, tag="semhc")
        nc.vector.tensor_add(hct[:], zT[1][kk][:], diff[:])
        hc_cb(kk, hct)


def make_cfg(Nd, Nc):
    ndp = -(-Nd // 1024) * 1024
    ncp = -(-Nc // 1024) * 1024
    return dict(Nd=Nd, Nc=Nc, NDP=ndp, NCP=ncp, DBLK=ndp // 8,
                CBLK=ncp // 8, HALF=ndp // 2, n_cores=8)


def build_program(sched, cfg, legalize=True):
    nco = cfg['n_cores']
    DBLK, CBLK, HALF = cfg['DBLK'], cfg['CBLK'], cfg['HALF']
    NDP, NCP = cfg['NDP'], cfg['NCP']
    DD_T, CT, DC2_T = DBLK // P, CBLK // P, NCP // P

    nc = bacc.Bacc(None)
    d = {}
    def inp(name, shape, dt=F32):
        d[name] = nc.declare_dram_parameter(name, list(shape), dt, isOutput=False)
        return d[name]

    featDT = inp('featDT', (2, P, NDP))
    featCT = inp('featCT', (2, P, NCP))
    featDTloc = inp('featDTloc', (2, P, DBLK))
    iota_in = inp('iota', (P, P))
    ident_in = inp('ident', (P, P))
    WDD = inp('WDD', (2, P, ROWW)); WerDD = inp('WerDD', (2, P, 64))
    WDC = inp('WDC', (2, P, ROWW)); WCC1 = inp('WCC1', (2, P, ROWW))
    WDC2 = inp('WDC2', (2, P, ROWW)); WCC2 = inp('WCC2', (2, P, ROWW))
    bias5 = inp('bias5', (5, FD))
    semW1 = inp('semW1', (2, 2, P, P)); semb1 = inp('semb1', (2, P, 1))
    semW2 = inp('semW2', (2, P, 1))
    dnnW1 = inp('dnnW1', (2, P, 32)); dnnb1 = inp('dnnb1', (32, 1))
    dnnW2 = inp('dnnW2', (32, 16)); dnnb2 = inp('dnnb2', (16, 1))
    dnnW3 = inp('dnnW3', (16, 1)); dnnb3 = inp('dnnb3', (1, 1))
    streams = {}
    for r, nchs in sched.items():
        tot = int(nchs.sum())
        streams[r] = (inp(f'{r}_src', (P, tot * 8), I16),
                      inp(f'{r}_er', (P, tot * 8), I16),
                      inp(f'{r}_dl', (P, tot)))
    out = nc.declare_dram_parameter('out', [1, CBLK], F32, isOutput=True)

    with tile.TileContext(nc) as tc:
        with tc.tile_pool(name="const", bufs=1) as cpool, \
             tc.tile_pool(name="sb", bufs=3) as sb, \
             tc.tile_pool(name="gather", bufs=2) as gb, \
             tc.tile_pool(name="ps", bufs=2, space="PSUM") as ps, \
             tc.tile_pool(name="dram", bufs=1, space="DRAM") as dr:
            nc.gpsimd.load_library(library_config.mlp)

            # ---- constants in SBUF
            consts = {}
            it = cpool.tile([P, P], F32); nc.sync.dma_start(it[:], iota_in[:])
            consts['iota'] = it
            idt = cpool.tile([P, P], F32)
            nc.sync.dma_start(idt[:], ident_in[:])
            consts['ident'] = idt
            ones1 = cpool.tile([1, P], F32); nc.vector.memset(ones1[:], 1.0)
            consts['ones1'] = ones1
            def wtile(ap, ncols, nm, n=2):
                ts = []
                for k in range(n):
                    t_ = cpool.tile([P, ncols], F32, tag=f"w_{nm}_{k}")
                    nc.sync.dma_start(t_[:], ap[k, :, :ncols])
                    ts.append(t_)
                return tuple(ts)
            wt = {'WDD': wtile(WDD, ROWW, 'dd'), 'WerDD': wtile(WerDD, 64, 'erdd'),
                  'WDC': wtile(WDC, ROWW, 'dc'), 'WCC1': wtile(WCC1, ROWW, 'cc1'),
                  'WDC2': wtile(WDC2, ROWW, 'dc2'), 'WCC2': wtile(WCC2, ROWW, 'cc2')}
            btiles = []
            for r in range(5):
                bt = cpool.tile([P, FD], F32, tag=f"bias_{r}")
                nc.sync.dma_start(bt[:], bias5[r:r + 1, :].to_broadcast([P, FD]))
                btiles.append(bt)
            consts['semW1'] = []
            consts['semb1'] = []
            consts['semW2'] = []
            for l in range(2):
                t0 = cpool.tile([P, P], F32, tag=f"sw1a{l}")
                nc.sync.dma_start(t0[:], semW1[l, 0])
                t1 = cpool.tile([P, P], F32, tag=f"sw1b{l}")
                nc.sync.dma_start(t1[:], semW1[l, 1])
                consts['semW1'].append((t0, t1))
                tb = cpool.tile([P, 1], F32, tag=f"sb1{l}")
                nc.sync.dma_start(tb[:], semb1[l])
                consts['semb1'].append(tb)
                tw = cpool.tile([P, 1], F32, tag=f"sw2{l}")
                nc.sync.dma_start(tw[:], semW2[l])
                consts['semW2'].append(tw)
            dW1 = wtile(dnnW1, 32, 'dnn1')
            dW2 = cpool.tile([32, 16], F32); nc.sync.dma_start(dW2[:], dnnW2[:])
            dW3 = cpool.tile([16, 1], F32); nc.sync.dma_start(dW3[:], dnnW3[:])
            db1 = cpool.tile([32, 1], F32); nc.sync.dma_start(db1[:], dnnb1[:])
            db2 = cpool.tile([16, 1], F32); nc.sync.dma_start(db2[:], dnnb2[:])
            db3 = cpool.tile([1, 1], F32); nc.sync.dma_start(db3[:], dnnb3[:])

            # ---- internal DRAM
            tabDD = dr.tile([NDP, ROWW], F32)
            tabDC = dr.tile([NDP, ROWW], F32)
            erDD = dr.tile([DBLK, 64], F32)
            tabCC1 = dr.tile([NCP, ROWW], F32)
            tabDC2 = dr.tile([DBLK, ROWW], F32)
            tabCC2 = dr.tile([NCP, ROWW], F32)
            hd1cT = dr.tile([2 * P, DBLK], F32)
            oDC1T = dr.tile([2 * P, CBLK], F32)
            oCC1T = dr.tile([2 * P, CBLK], F32)
            oDC2T = dr.tile([2 * P, CBLK], F32)
            oCC2T = dr.tile([2 * P, CBLK], F32)
            hc1T = dr.tile([2 * P, CBLK], F32)
            hc1T_ag = dr.tile([nco * 2 * P, CBLK], F32, addr_space="Shared")
            Upart = dr.tile([NCP, 264], F32)
            Ua2a = dr.tile([NCP, 264], F32)

            # ---- L1 projections (replicated tables + local er)
            _proj_pass(nc, sb, ps, featDT, NDP // P,
                       [(tabDD.opt(), ROWW, 'WDD'), (tabDC.opt(), ROWW, 'WDC')], wt)
            _proj_pass(nc, sb, ps, featCT, NCP // P,
                       [(tabCC1.opt(), ROWW, 'WCC1')], wt)
            _proj_pass(nc, sb, ps, featDTloc, DD_T,
                       [(erDD.opt(), 64, 'WerDD')], wt)

            # ---- L1 dd edge phase -> hd1cT (transposed store)
            def dd_epi(t, pp, sb_):
                o = sb_.tile([P, FD], F32, tag="oed")
                _normalize_elu(nc, sb_, pp, btiles[0], o)
                _transpose_store(nc, sb_, ps, consts, o, hd1cT.opt(), t * P)
            _edge_phase(nc, gb, ps, consts,
                        [tabDD.opt()[0:HALF, :], tabDD.opt()[HALF:NDP, :]],
                        erDD.opt()[:, :], streams['dd'], sched['dd'], dd_epi, 0,
                        er_step=64)

            # ---- tabDC2 projection from hd1cT (local drugs)
            _proj_pass(nc, sb, ps, hd1cT.opt().rearrange("(a p) n -> a p n", p=P),
                       DD_T, [(tabDC2.opt(), ROWW, 'WDC2')], wt)

            # ---- L1 dc edge phase -> oDC1T
            def dc1_epi(t, pp, sb_):
                o = sb_.tile([P, FD], F32, tag="oed")
                _normalize_elu(nc, sb_, pp, btiles[1], o)
                _transpose_store(nc, sb_, ps, consts, o, oDC1T.opt(), t * P)
            _edge_phase(nc, gb, ps, consts,
                        [tabDC.opt()[0:HALF, :], tabDC.opt()[HALF:NDP, :]],
                        tabCC1.opt()[:, ERG:ERG + 64], streams['dc1'],
                        sched['dc1'], dc1_epi, 16)

            # ---- L1 cc edge phase -> oCC1T
            def cc1_epi(t, pp, sb_):
                o = sb_.tile([P, FD], F32, tag="oed")
                _normalize_elu(nc, sb_, pp, btiles[2], o)
                _transpose_store(nc, sb_, ps, consts, o, oCC1T.opt(), t * P)
            _edge_phase(nc, gb, ps, consts, [tabCC1.opt()[:, :]],
                        tabCC1.opt()[:, ERG:ERG + 64], streams['cc1'],
                        sched['cc1'], cc1_epi, 8)

            # ---- sem attention L1 -> hc1T, then AllGather
            for t in range(CT):
                def hc_cb(kk, hct, t=t):
                    nc.sync.dma_start(hc1T.opt()[kk * P:(kk + 1) * P,
                                                 t * P:(t + 1) * P], hct[:])
                _sem_combine(nc, sb, ps, consts, oDC1T.opt(), oCC1T.opt(),
                             0, t, hc_cb)
            nc.gpsimd.collective_compute(
                "AllGather", AluOp.bypass,
                replica_groups=[list(range(nco))],
                ins=[hc1T.opt()], outs=[hc1T_ag.opt()])

            # ---- tabCC2 projection (replicated, from AllGathered hc1T)
            agv = hc1T_ag.opt().rearrange("(c a p) n -> c a p n", a=2, p=P)
            for t in range(DC2_T):
                c_, j = t // CT, t % CT
                lh0 = sb.tile([P, P], F32, tag="projlh")
                lh1 = sb.tile([P, P], F32, tag="projlh")
                nc.sync.dma_start(lh0[:], agv[c_, 0, :, j * P:(j + 1) * P])
                nc.sync.dma_start(lh1[:], agv[c_, 1, :, j * P:(j + 1) * P])
                pp = ps.tile([P, ROWW], F32, space="PSUM", tag="projps")
                w0, w1 = wt['WCC2']
                nc.tensor.matmul(pp[:], lhsT=lh0[:], rhs=w0[:], start=True, stop=False)
                nc.tensor.matmul(pp[:], lhsT=lh1[:], rhs=w1[:], start=False, stop=True)
                ot = sb.tile([P, ROWW], F32, tag="projout")
                nc.scalar.copy(ot[:], pp[:])
                nc.sync.dma_start(tabCC2.opt()[t * P:(t + 1) * P, :], ot[:])

            # ---- L2 dc edge phase (by src) -> raw partial U
            def dc2_epi(t, pp, sb_):
                o = sb_.tile([P, 264], F32, tag="oraw")
                nc.scalar.copy(o[:], pp[:])
                nc.sync.dma_start(Upart.opt()[t * P:(t + 1) * P, :], o[:])
            _edge_phase(nc, gb, ps, consts, [tabDC2.opt()[:, :]],
                        tabCC2.opt()[:, ERG:ERG + 64], streams['dc2'],
                        sched['dc2'], dc2_epi, 16)

            # ---- AllToAll partial U; local reduce + normalize -> oDC2T
            nc.gpsimd.collective_compute(
                "AllToAll", AluOp.bypass,
                replica_groups=[list(range(nco))],
                ins=[Upart.opt()], outs=[Ua2a.opt()])
            a2av = Ua2a.opt().rearrange("(c r) n -> c r n", c=nco)
            for t in range(CT):
                acc = sb.tile([P, 264], F32, tag="acc")
                tmp = sb.tile([P, 264], F32, tag="acct")
                nc.sync.dma_start(acc[:], a2av[0, t * P:(t + 1) * P, :])
                for j in range(1, nco):
                    nc.sync.dma_start(tmp[:], a2av[j, t * P:(t + 1) * P, :])
                    nc.vector.tensor_add(acc[:], acc[:], tmp[:])
                    tmp = sb.tile([P, 264], F32, tag="acct")
                o = sb.tile([P, FD], F32, tag="oed")
                _normalize_elu_sbuf(nc, sb, acc, btiles[3], o)
                _transpose_store(nc, sb, ps, consts, o, oDC2T.opt(), t * P)

            # ---- L2 cc edge phase -> oCC2T
            def cc2_epi(t, pp, sb_):
                o = sb_.tile([P, FD], F32, tag="oed")
                _normalize_elu(nc, sb_, pp, btiles[4], o)
                _transpose_store(nc, sb_, ps, consts, o, oCC2T.opt(), t * P)
            _edge_phase(nc, gb, ps, consts, [tabCC2.opt()[:, :]],
                        tabCC2.opt()[:, ERG:ERG + 64], streams['cc2'],
                        sched['cc2'], cc2_epi, 8)

            # ---- sem attention L2 + MLP head
            for t in range(CT):
                hct_tiles = {}
                def hc2_cb(kk, hct, bag=hct_tiles):
                    bag[kk] = hct
                _sem_combine(nc, sb, ps, consts, oDC2T.opt(), oCC2T.opt(),
                             1, t, hc2_cb)
                h1p = ps.tile([32, P], F32, space="PSUM", tag="aux")
                nc.tensor.matmul(h1p[:], lhsT=dW1[0][:], rhs=hct_tiles[0][:],
                                 start=True, stop=False)
                nc.tensor.matmul(h1p[:], lhsT=dW1[1][:], rhs=hct_tiles[1][:],
                                 start=False, stop=True)
                h1 = sb.tile([32, P], F32, tag="mlph1")
                nc.vector.scalar_tensor_tensor(
                    out=h1[:], in0=h1p[:], scalar=1.0, in1=db1[:].to_broadcast([32, P]),
                    op0=AluOp.mult, op1=AluOp.add)
                nc.vector.scalar_tensor_tensor(
                    out=h1[:], in0=h1[:], scalar=0.01, in1=h1[:],
                    op0=AluOp.mult, op1=AluOp.max)
                h2p = ps.tile([16, P], F32, space="PSUM", tag="aux")
                nc.tensor.matmul(h2p[:], lhsT=dW2[:], rhs=h1[:], start=True, stop=True)
                h2 = sb.tile([16, P], F32, tag="mlph2")
                nc.vector.scalar_tensor_tensor(
                    out=h2[:], in0=h2p[:], scalar=1.0, in1=db2[:].to_broadcast([16, P]),
                    op0=AluOp.mult, op1=AluOp.add)
                nc.vector.scalar_tensor_tensor(
                    out=h2[:], in0=h2[:], scalar=0.01, in1=h2[:],
                    op0=AluOp.mult, op1=AluOp.max)
                h3p = ps.tile([1, P], F32, space="PSUM", tag="aux")
                nc.tensor.matmul(h3p[:], lhsT=dW3[:], rhs=h2[:], start=True, stop=True)
                h3 = sb.tile([1, P], F32, tag="mlph3")
                nc.vector.tensor_scalar(h3[:], h3p[:], db3[:], None, op0=AluOp.add)
                nc.sync.dma_start(out[0:1, t * P:(t + 1) * P], h3[:])

    nc.compile()
    if legalize:
        legalize_waits(nc)
    return nc


def _normalize_elu_sbuf(nc, sb, acc, bias_tile, out_tile):
    """Same as _normalize_elu but source is an SBUF tile [P,264]."""
    den = sb.tile([P, 8], F32, tag="den")
    nc.vector.tensor_scalar_max(den[:], acc[:, FD:FD + 8], 1e-30)
    rec = sb.tile([P, 8], F32, tag="rec")
    nc.vector.reciprocal(rec[:], den[:])
    x = sb.tile([P, FD], F32, tag="xnrm")
    nc.vector.tensor_tensor(
        out=x[:].rearrange("p (h d) -> p h d", h=H),
        in0=acc[:, :FD].rearrange("p (h d) -> p h d", h=H),
        in1=rec[:, :, None].to_broadcast([P, H, 32]), op=AluOp.mult)
    nc.vector.tensor_add(x[:], x[:], bias_tile[:])
    ex = sb.tile([P, FD], F32, tag="eluex")
    nc.scalar.activation(ex[:], x[:], Act.Exp)
    nc.scalar.activation(ex[:], ex[:], Act.Relu, bias=1.0, scale=-1.0)
    nc.scalar.activation(out_tile[:], x[:], Act.Relu)
    nc.vector.tensor_sub(out_tile[:], out_tile[:], ex[:])


# --------------------------------------------------------------------------
# entry point
# --------------------------------------------------------------------------

_CACHE = {}


def kernel(**inputs):
    cfg = make_cfg(inputs['feat_drug'].shape[0], inputs['feat_cell'].shape[0])
    sched, in_maps = host_prep(inputs, cfg)
    key = tuple(int(x) for s in sched.values() for x in s.flatten())
    if key not in _CACHE:
        _CACHE[key] = build_program(sched, cfg)
    nc = _CACHE[key]
    from concourse.bass_utils import run_bass_kernel_spmd
    res = run_bass_kernel_spmd(nc, in_maps, list(range(cfg['n_cores'])))
    pieces = [res.results[c]['out'][0] for c in range(cfg['n_cores'])]
    full = np.concatenate([p[:cfg['CBLK']] for p in pieces])[:cfg['Nc']]
    return full.reshape(-1, 1).astype(np.float32)



# revision 13
# speedup vs baseline: 1.3385x; 1.3385x over previous
"""Trainium2 Bass kernel v2 for the 2-layer heterogeneous GAT (drug/cell).

Strategy (8 NeuronCores, SPMD single program), changes vs v1:
  - All five edge phases partitioned by DST block; dc2 now dst-partitioned
    too, fed by an AllGather of hd1 (bf16) instead of AllToAll partials.
  - bf16 gather tables, 384-col rows (768B, %256B) holding [fs 256 | el 8].
  - er is never gathered: per-dst-tile er vectors [128, 8] are stashed in
    SBUF (projected from local features / hc1 tiles) and broadcast to edges
    with a one-hot matmul (lhsT = S2[dst -> edge]).
  - Edge phase batches DVE work per group of <=16 chunks: one is_equal for
    all S chunks, one for S2 (dst-row stream replicated via DMA broadcast),
    one fused exp/leaky pipeline, one rhs build.
  - Gathers up to 1024 idx per call (SWDGE ring limit is < 2048).
  - Projections batched: 8 tiles per DMA load/store, psum->SBUF copies
    alternate ACT/DVE.
"""
import sys
sys.path.insert(0, '/opt/trn_rl_repo')
import numpy as np
import ml_dtypes

import concourse.bacc as bacc
import concourse.tile as tile
from concourse import mybir, library_config

F32 = mybir.dt.float32
BF16 = mybir.dt.bfloat16
FP8 = mybir.dt.float8e4
I16 = mybir.dt.int16
BF = ml_dtypes.bfloat16
F8 = mybir.dt.np(FP8)
P = 128
H = 8
FD = 256          # feature dim
TW = 384          # gather-table row width (bf16) = 768B; payload 264
GSZ = 16          # chunks per batched edge group
GCH = 8           # chunks per dma_gather call (1024 idx; 2048 hangs)
AluOp = mybir.AluOpType
Act = mybir.ActivationFunctionType

N_CORES = 8


def legalize_waits(nc):
    """Split multi-wait instructions into wait-carrying NOP chains."""
    n_split = 0
    for fn in nc.m.functions:
        for bb in fn.blocks:
            insts = bb.instructions
            new = []
            changed = False
            for inst in insts:
                si = inst.sync_info
                waits = list(si.on_wait) if si is not None else []
                cap = 2 if isinstance(inst, mybir.InstEventSemaphore) else 1
                if len(waits) > cap:
                    keep = waits[-cap:]
                    for w in waits[:-cap]:
                        nop = mybir.InstNoOp(
                            name=nc.get_next_instruction_name(),
                            engine=inst.engine,
                            sync_info=mybir.SyncInfo(on_wait=[w], on_update=[]),
                            bass_nofuse=True,
                        )
                        new.append(nop)
                        n_split += 1
                    inst.sync_info = mybir.SyncInfo(
                        on_wait=keep, on_update=list(si.on_update))
                    changed = True
                new.append(inst)
            if changed:
                bb.instructions = new
    return n_split


def make_cfg(Nd, Nc):
    ndp = -(-Nd // 1024) * 1024
    ncp = -(-Nc // 1024) * 1024
    return dict(Nd=Nd, Nc=Nc, NDP=ndp, NCP=ncp, DBLK=ndp // 8,
                CBLK=ncp // 8, HALF=ndp // 2, n_cores=8)


# --------------------------------------------------------------------------
# host-side prep
# --------------------------------------------------------------------------

def _fold_weights(ip):
    def wel(W, a):  # W [256,256], a [H,D] -> [256,H]
        return (np.asarray(W, np.float32).reshape(FD, H, -1)
                * np.asarray(a, np.float32)[None]).sum(-1)
    Wsrc = np.asarray(ip['Wsrc'], np.float32)
    Wdst = np.asarray(ip['Wdst_dc'], np.float32)
    al = np.asarray(ip['attn_l'], np.float32)
    ar = np.asarray(ip['attn_r'], np.float32)

    def w264(W, a):
        return np.concatenate([W, wel(W, a)], 1)  # [256, 264]

    def bftile(a, ncols):  # [256, ncols] -> [2, 128, ncols] bf16
        return np.ascontiguousarray(a.reshape(2, P, ncols)).astype(BF)

    out = {}
    out['WDD'] = bftile(w264(Wsrc[0, 0], al[0, 0]), 264)
    out['WDC'] = bftile(w264(Wsrc[0, 1], al[0, 1]), 264)
    out['WCC1'] = bftile(w264(Wsrc[0, 2], al[0, 2]), 264)
    out['WDC2'] = bftile(w264(Wsrc[1, 1], al[1, 1]), 264)
    out['WCC2'] = bftile(w264(Wsrc[1, 2], al[1, 2]), 264)
    out['WerDD'] = bftile(wel(Wsrc[0, 0], ar[0, 0]), 8)
    out['WerC1'] = bftile(np.concatenate(
        [wel(Wdst[0], ar[0, 1]), wel(Wsrc[0, 2], ar[0, 2])], 1), 16)
    out['WerC2'] = bftile(np.concatenate(
        [wel(Wdst[1], ar[1, 1]), wel(Wsrc[1, 2], ar[1, 2])], 1), 16)
    out['bias5'] = np.stack([ip['gat_bias'][0, 0], ip['gat_bias'][0, 1],
                             ip['gat_bias'][0, 2], ip['gat_bias'][1, 1],
                             ip['gat_bias'][1, 2]]).astype(np.float32)
    out['semW1'] = np.stack([np.asarray(ip['sem_W1'][l, 1], np.float32)
                             .reshape(2, P, P) for l in range(2)]).astype(BF)
    out['semb1'] = np.stack([np.asarray(ip['sem_b1'][l, 1], np.float32)
                             .reshape(P, 1) for l in range(2)]).astype(np.float32)
    out['semW2'] = np.stack([np.asarray(ip['sem_W2'][l, 1], np.float32)
                             .reshape(P, 1) for l in range(2)]).astype(BF)
    out['dnnW1'] = np.asarray(ip['dnn_W1'], np.float32).reshape(2, P, 32).astype(BF)
    out['dnnb1'] = np.asarray(ip['dnn_b1'], np.float32).reshape(32, 1)
    out['dnnW2'] = np.asarray(ip['dnn_W2'], np.float32).astype(BF)
    out['dnnb2'] = np.asarray(ip['dnn_b2'], np.float32).reshape(16, 1)
    out['dnnW3'] = np.asarray(ip['dnn_W3'], np.float32).astype(BF)
    out['dnnb3'] = np.asarray(ip['dnn_b3'], np.float32).reshape(1, 1)
    return out


def _wrap16(vals):
    n = len(vals)
    out = np.zeros((P, n // 16), np.int16)
    a = np.asarray(vals, np.int16).reshape(-1, 16).T
    for g in range(8):
        out[g * 16:(g + 1) * 16, :] = a
    return out


def _prep_rel(src, dst, nco, *, blk, n_tiles, halves):
    """Per-core edge schedule, dst-partitioned.

    Returns nch [T, n_halves] (equalized over cores) and per-core streams:
    src idx wrapped i16 [P, tot*8], plus host-built one-hot scatter
    matrices in fp8: S [P, tot*128] (S[p, k*128+d] = dl[p,k]==d) and
    S2 [P, tot*128] (S2[p, k*128+e] = dl[e,k]==p).
    """
    src = np.asarray(src, np.int64)
    dst = np.asarray(dst, np.int64)
    per_core = []
    for c in range(nco):
        m = (dst >= c * blk) & (dst < (c + 1) * blk)
        dl = dst[m] - c * blk
        s = src[m]
        tid = dl // P
        per_core.append((s, dl % P, tid))
    nh = 2 if halves else 1
    nch = np.zeros((n_tiles, nh), np.int64)
    buckets = []
    for c in range(nco):
        s, dloc, tid = per_core[c]
        bk = {}
        for t in range(n_tiles):
            mt = tid == t
            st, dt_ = s[mt], dloc[mt]
            if halves:
                m0 = st < halves
                groups = [(st[m0], dt_[m0]), (st[~m0] - halves, dt_[~m0])]
            else:
                groups = [(st, dt_)]
            bk[t] = groups
            for h, (gs, gd) in enumerate(groups):
                nch[t, h] = max(nch[t, h], (len(gs) + P - 1) // P)
        buckets.append(bk)
    nch = np.maximum(nch, 1)
    tot = int(nch.sum())
    rng = np.arange(P, dtype=np.int64)
    srcs, Ss, S2s = [], [], []
    for c in range(nco):
        bk = buckets[c]
        sw = np.zeros((P, tot * 8), np.int16)
        dall = np.full((tot, P), -1, np.int64)   # [chunk, edge] local dst
        col = 0
        for t in range(n_tiles):
            for h, (gs, gd) in enumerate(bk[t]):
                n = int(nch[t, h]) * P
                a = np.zeros(n, np.int64)
                a[:len(gs)] = gs
                assert a.max(initial=0) < 32768
                sw[:, col * 8:(col + int(nch[t, h])) * 8] = _wrap16(a)
                dpad = np.full(n, -1, np.int64)
                dpad[:len(gd)] = gd
                dall[col:col + int(nch[t, h])] = dpad.reshape(-1, P)
                col += int(nch[t, h])
        # S[p, k, d] = (dall[k, p] == d);  S2[p, k, e] = (dall[k, e] == p)
        S = (dall.T[:, :, None] == rng[None, None, :]).astype(F8)
        S2 = (dall[None, :, :] == rng[:, None, None]).astype(F8)
        srcs.append(sw)
        Ss.append(np.ascontiguousarray(S.reshape(P, tot * P)))
        S2s.append(np.ascontiguousarray(S2.reshape(P, tot * P)))
    return dict(nch=nch, src=srcs, S=Ss, S2=S2s, tot=tot)


def host_prep(ip, cfg):
    W = _fold_weights(ip)
    nco = cfg['n_cores']
    DBLK, CBLK, HALF = cfg['DBLK'], cfg['CBLK'], cfg['HALF']
    NDP, NCP = cfg['NDP'], cfg['NCP']
    DD_T, CT = DBLK // P, CBLK // P

    dd = _prep_rel(ip['src_dd'], ip['dst_dd'], nco, blk=DBLK,
                   n_tiles=DD_T, halves=HALF)
    dc = _prep_rel(ip['src_dc'], ip['dst_dc'], nco, blk=CBLK,
                   n_tiles=CT, halves=HALF)
    cc = _prep_rel(ip['src_cc'], ip['dst_cc'], nco, blk=CBLK,
                   n_tiles=CT, halves=None)

    featD = np.zeros((NDP, FD), np.float32)
    featD[:cfg['Nd']] = np.asarray(ip['feat_drug'], np.float32)
    featC = np.zeros((NCP, FD), np.float32)
    featC[:cfg['Nc']] = np.asarray(ip['feat_cell'], np.float32)
    featDT = np.ascontiguousarray(featD.T).reshape(2, P, NDP).astype(BF)
    featCT = np.ascontiguousarray(featC.T).reshape(2, P, NCP).astype(BF)

    identB = np.eye(P, dtype=np.float32).astype(BF)

    base = dict(featDT=featDT, featCT=featCT, identB=identB, **W)
    in_maps = []
    for c in range(nco):
        m = dict(base)
        m['featDTloc'] = np.ascontiguousarray(
            featD[c * DBLK:(c + 1) * DBLK].T).reshape(2, P, DBLK).astype(BF)
        m['featCTloc'] = np.ascontiguousarray(
            featC[c * CBLK:(c + 1) * CBLK].T).reshape(2, P, CBLK).astype(BF)
        for r, d in (('dd', dd), ('dc', dc), ('cc', cc)):
            m[f'{r}_src'] = d['src'][c]
            m[f'{r}_S'] = d['S'][c]
            m[f'{r}_S2'] = d['S2'][c]
        in_maps.append(m)
    sched = dict(dd=dd['nch'], dc=dc['nch'], cc=cc['nch'])
    return sched, in_maps


# --------------------------------------------------------------------------
# device program
# --------------------------------------------------------------------------

def _tile_segments(nch_row):
    """Per-tile gather segments [(start_col, n, half)], respecting half
    boundaries, GSZ group windows, and GCH call caps."""
    bounds = []
    off = 0
    for h, nh in enumerate(nch_row):
        bounds.append((off, off + int(nh), h))
        off += int(nh)
    tnch = off
    segs = []
    for g0 in range(0, tnch, GSZ):
        g1 = min(g0 + GSZ, tnch)
        for (h0, h1, h) in bounds:
            s0 = max(g0, h0)
            s1 = min(g1, h1)
            for b in range(s0, s1, GCH):
                segs.append((b, min(GCH, s1 - b), h))
    return segs, tnch


def build_program(sched, cfg, legalize=True):
    nco = cfg['n_cores']
    DBLK, CBLK, HALF = cfg['DBLK'], cfg['CBLK'], cfg['HALF']
    NDP, NCP = cfg['NDP'], cfg['NCP']
    DD_T, CT = DBLK // P, CBLK // P
    NDT, NCT = NDP // P, NCP // P

    nc = bacc.Bacc(None)
    d = {}
    def inp(name, shape, dt=BF16):
        d[name] = nc.declare_dram_parameter(name, list(shape), dt,
                                            isOutput=False)
        return d[name]

    featDT = inp('featDT', (2, P, NDP))
    featCT = inp('featCT', (2, P, NCP))
    featDTloc = inp('featDTloc', (2, P, DBLK))
    featCTloc = inp('featCTloc', (2, P, CBLK))
    identB_in = inp('identB', (P, P))
    Wmain = {k: inp(k, (2, P, 264)) for k in
             ('WDD', 'WDC', 'WCC1', 'WDC2', 'WCC2')}
    WerDD = inp('WerDD', (2, P, 8))
    WerC1 = inp('WerC1', (2, P, 16))
    WerC2 = inp('WerC2', (2, P, 16))
    bias5 = inp('bias5', (5, FD), F32)
    semW1 = inp('semW1', (2, 2, P, P))
    semb1 = inp('semb1', (2, P, 1), F32)
    semW2 = inp('semW2', (2, P, 1))
    dnnW1 = inp('dnnW1', (2, P, 32)); dnnb1 = inp('dnnb1', (32, 1), F32)
    dnnW2 = inp('dnnW2', (32, 16)); dnnb2 = inp('dnnb2', (16, 1), F32)
    dnnW3 = inp('dnnW3', (16, 1)); dnnb3 = inp('dnnb3', (1, 1), F32)
    streams = {}
    for r, nchs in sched.items():
        tot = int(nchs.sum())
        streams[r] = (inp(f'{r}_src', (P, tot * 8), I16),
                      inp(f'{r}_S', (P, tot * P), FP8),
                      inp(f'{r}_S2', (P, tot * P), FP8))
    out = nc.declare_dram_parameter('out', [1, CBLK], F32, isOutput=True)

    with tile.TileContext(nc) as tc:
        with tc.tile_pool(name="const", bufs=1) as cpool, \
             tc.tile_pool(name="sb", bufs=3) as sb, \
             tc.tile_pool(name="pl", bufs=2) as pl, \
             tc.tile_pool(name="gb", bufs=2) as gb, \
             tc.tile_pool(name="eb", bufs=2) as eb, \
             tc.tile_pool(name="ps", bufs=2, space="PSUM") as ps, \
             tc.tile_pool(name="psm", bufs=2, space="PSUM") as psm, \
             tc.tile_pool(name="dram", bufs=1, space="DRAM") as dr:
            nc.gpsimd.load_library(library_config.mlp)

            # ---- constants in SBUF
            C = {}
            def cload(name, ap, shape, dt=BF16):
                t_ = cpool.tile(list(shape), dt, tag=f"c_{name}")
                nc.sync.dma_start(t_[:], ap)
                return t_
            C['ident'] = cload('ident', identB_in[:], (P, P))
            ones1 = cpool.tile([1, P], BF16)
            nc.vector.memset(ones1[:], 1.0)
            wt = {}
            for k in Wmain:
                wt[k] = tuple(cload(f'{k}{j}', Wmain[k][j], (P, 264))
                              for j in range(2))
            werDD = tuple(cload(f'werDD{j}', WerDD[j], (P, 8))
                          for j in range(2))
            werC1 = tuple(cload(f'werC1{j}', WerC1[j], (P, 16))
                          for j in range(2))
            werC2 = tuple(cload(f'werC2{j}', WerC2[j], (P, 16))
                          for j in range(2))
            btiles = [cload(f'bias{r}', bias5[r:r + 1, :].to_broadcast([P, FD]),
                            (P, FD), F32) for r in range(5)]
            sw1 = [tuple(cload(f'sw1_{l}{j}', semW1[l, j], (P, P))
                         for j in range(2)) for l in range(2)]
            sb1 = [cload(f'sb1_{l}', semb1[l], (P, 1), F32) for l in range(2)]
            sw2 = [cload(f'sw2_{l}', semW2[l], (P, 1)) for l in range(2)]
            dW1 = tuple(cload(f'dW1{j}', dnnW1[j], (P, 32)) for j in range(2))
            dW2 = cload('dW2', dnnW2[:], (32, 16))
            dW3 = cload('dW3', dnnW3[:], (16, 1))
            db1 = cload('db1', dnnb1[:], (32, 1), F32)
            db2 = cload('db2', dnnb2[:], (16, 1), F32)
            db3 = cload('db3', dnnb3[:], (1, 1), F32)
            # resident local features (for er projections)
            fdl = tuple(cload(f'fdl{j}', featDTloc[j], (P, DBLK))
                        for j in range(2))
            fcl = tuple(cload(f'fcl{j}', featCTloc[j], (P, CBLK))
                        for j in range(2))
            # resident edge streams
            SI = {}
            for r in sched:
                tot = int(sched[r].sum())
                SI[r] = cload(f'si_{r}', streams[r][0][:], (P, tot * 8), I16)
            # er stashes
            erDDs = cpool.tile([P, DD_T * 8], BF16, tag="erDDs")
            erC1s = cpool.tile([P, CT * 16], BF16, tag="erC1s")
            erC2s = cpool.tile([P, CT * 16], BF16, tag="erC2s")

            # ---- internal DRAM
            tabDD = dr.tile([NDP, TW], BF16)
            tabDC = dr.tile([NDP, TW], BF16)
            tabCC1 = dr.tile([NCP, TW], BF16)
            tabDC2 = dr.tile([NDP, TW], BF16)
            tabCC2 = dr.tile([NCP, TW], BF16)
            hd1T = dr.tile([2 * P, DBLK], BF16)
            hd1T_ag = dr.tile([nco * 2 * P, DBLK], BF16, addr_space="Shared")
            oDC1T = dr.tile([2 * P, CBLK], BF16)
            oCC1T = dr.tile([2 * P, CBLK], BF16)
            oDC2T = dr.tile([2 * P, CBLK], BF16)
            oCC2T = dr.tile([2 * P, CBLK], BF16)
            hc1T = dr.tile([2 * P, CBLK], BF16)
            hc1T_ag = dr.tile([nco * 2 * P, CBLK], BF16, addr_space="Shared")

            # ---- er stash projections (local features)
            def er_stash(fres, wpair, stash, n_tiles, ncols):
                BT = min(8, n_tiles) if ncols == 8 else n_tiles
                for t0 in range(0, n_tiles, BT):
                    bt = min(BT, n_tiles - t0)
                    pp = ps.tile([P, bt * ncols], F32, tag="projps")
                    for i in range(bt):
                        tl = (t0 + i) * P
                        nc.tensor.matmul(pp[:, i * ncols:(i + 1) * ncols],
                                         lhsT=fres[0][:, tl:tl + P],
                                         rhs=wpair[0][:], start=True, stop=False)
                        nc.tensor.matmul(pp[:, i * ncols:(i + 1) * ncols],
                                         lhsT=fres[1][:, tl:tl + P],
                                         rhs=wpair[1][:], start=False, stop=True)
                    nc.scalar.copy(stash[:, t0 * ncols:(t0 + bt) * ncols], pp[:])
            er_stash(fdl, werDD, erDDs, DD_T, 8)
            er_stash(fcl, werC1, erC1s, CT, 16)

            # ---- batched projection pass (multi-job: shared lhs loads)
            def proj(lhs_ap_fn, n_tiles, jobs, BT=8):
                """lhs_ap_fn(k, c0, n) -> DRAM AP [P, n] for k-chunk cols.
                jobs: list of (wpair, tab)."""
                for t0 in range(0, n_tiles, BT):
                    bt = min(BT, n_tiles - t0)
                    lh = pl.tile([P, 2, bt * P], BF16, tag="projlh")
                    nc.sync.dma_start(lh[:, 0, :], lhs_ap_fn(0, t0 * P, bt * P))
                    nc.sync.dma_start(lh[:, 1, :], lhs_ap_fn(1, t0 * P, bt * P))
                    for j, (wpair, tab) in enumerate(jobs):
                        ob = pl.tile([P, bt, 264], BF16, tag=f"projout{j}")
                        for i in range(bt):
                            pp = ps.tile([P, 264], F32, tag="projps")
                            nc.tensor.matmul(pp[:],
                                             lhsT=lh[:, 0, i * P:(i + 1) * P],
                                             rhs=wpair[0][:],
                                             start=True, stop=False)
                            nc.tensor.matmul(pp[:],
                                             lhsT=lh[:, 1, i * P:(i + 1) * P],
                                             rhs=wpair[1][:],
                                             start=False, stop=True)
                            nc.scalar.copy(ob[:, i, :], pp[:])
                        nc.sync.dma_start(
                            tab[t0 * P:(t0 + bt) * P, 0:264]
                            .rearrange("(t p) c -> p t c", p=P), ob[:])

            def dram_lhs(apx):
                return lambda k, c0, n: apx[k, :, c0:c0 + n]

            # L1 projections: tabDD+tabDC share feature loads
            proj(dram_lhs(featDT), NDT, [(wt['WDD'], tabDD.opt()),
                                         (wt['WDC'], tabDC.opt())])
            proj(dram_lhs(featCT), NCT, [(wt['WCC1'], tabCC1.opt())])

            # ---- edge phase
            def edge_phase(rel, tab_halves, er_fn, nchs, epilogue):
                si = SI[rel]
                S_dram, S2_dram = streams[rel][1], streams[rel][2]
                T = nchs.shape[0]
                col = 0
                for t in range(T):
                    segs, tnch = _tile_segments(nchs[t])
                    mps = psm.tile([P, 264], F32, tag="mainps")
                    er_t = er_fn(t)
                    for g0 in range(0, tnch, GSZ):
                        gn = min(GSZ, tnch - g0)
                        G = gb.tile([P, gn, TW], BF16, tag="G")
                        for (s0, sn, h) in segs:
                            if s0 < g0 or s0 >= g0 + gn:
                                continue
                            nidx = sn * P
                            nc.gpsimd.dma_gather(
                                G[:, s0 - g0:s0 - g0 + sn, :], tab_halves[h],
                                si[:, (col + s0) * 8:(col + s0 + sn) * 8],
                                nidx, nidx, TW)
                        S = eb.tile([P, gn * P], FP8, tag="S")
                        nc.sync.dma_start(
                            S[:], S_dram[:, (col + g0) * P:(col + g0 + gn) * P])
                        S2 = eb.tile([P, gn * P], FP8, tag="S2")
                        nc.sync.dma_start(
                            S2[:], S2_dram[:, (col + g0) * P:(col + g0 + gn) * P])
                        erps = ps.tile([P, gn * 8], F32, tag="aux")
                        for k in range(gn):
                            nc.tensor.matmul(
                                erps[:, k * 8:(k + 1) * 8],
                                lhsT=S2[:, k * P:(k + 1) * P], rhs=er_t,
                                start=True, stop=True, skip_group_check=True)
                        ww = eb.tile([P, gn, 8], F32, tag="ww")
                        nc.vector.tensor_tensor(
                            out=ww[:], in0=G[:, :, FD:FD + 8],
                            in1=erps[:].rearrange("p (a b) -> p a b", b=8),
                            op=AluOp.add)
                        nc.vector.scalar_tensor_tensor(
                            out=ww[:], in0=ww[:], scalar=0.2, in1=ww[:],
                            op0=AluOp.mult, op1=AluOp.max)
                        nc.scalar.activation(ww[:], ww[:], Act.Exp)
                        rhs = eb.tile([P, gn, 264], BF16, tag="rhs")
                        nc.vector.tensor_tensor(
                            out=rhs[:, :, 0:FD]
                                .rearrange("p a (h e) -> p a h e", h=H),
                            in0=G[:, :, 0:FD]
                                .rearrange("p a (h e) -> p a h e", h=H),
                            in1=ww[:, :, :, None].to_broadcast([P, gn, H, 32]),
                            op=AluOp.mult)
                        nc.vector.tensor_copy(rhs[:, :, FD:FD + 8], ww[:])
                        for k in range(gn):
                            nc.tensor.matmul(
                                mps[:], lhsT=S[:, k * P:(k + 1) * P],
                                rhs=rhs[:, k, :],
                                start=(g0 == 0 and k == 0),
                                stop=(g0 + gn == tnch and k == gn - 1),
                                skip_group_check=True)
                    epilogue(t, mps)
                    col += tnch

            def _norm_elu(pp, bias_tile, o_bf):
                den = sb.tile([P, 8], F32, tag="den")
                nc.vector.tensor_scalar_max(den[:], pp[:, FD:FD + 8], 1e-30)
                rec = sb.tile([P, 8], F32, tag="rec")
                nc.vector.reciprocal(rec[:], den[:])
                x = sb.tile([P, FD], F32, tag="xnrm")
                nc.vector.tensor_tensor(
                    out=x[:].rearrange("p (h e) -> p h e", h=H),
                    in0=pp[:, 0:FD].rearrange("p (h e) -> p h e", h=H),
                    in1=rec[:, :, None].to_broadcast([P, H, 32]),
                    op=AluOp.mult)
                nc.vector.tensor_add(x[:], x[:], bias_tile[:])
                ex = sb.tile([P, FD], F32, tag="eluex")
                nc.scalar.activation(ex[:], x[:], Act.Exp)
                nc.scalar.activation(ex[:], ex[:], Act.Relu, bias=1.0, scale=-1.0)
                xp = sb.tile([P, FD], F32, tag="elup")
                nc.scalar.activation(xp[:], x[:], Act.Relu)
                nc.vector.tensor_sub(o_bf[:], xp[:], ex[:])

            def _tstore(o_bf, dramT, t):
                ts = sb.tile([P, 2, P], BF16, tag="tpsb")
                for kk in range(2):
                    tp = ps.tile([P, P], BF16, space="PSUM", tag="aux")
                    nc.tensor.transpose(tp[:], o_bf[:, kk * P:(kk + 1) * P],
                                        C['ident'][:])
                    nc.scalar.copy(ts[:, kk, :], tp[:])
                nc.sync.dma_start(
                    dramT[:, t * P:(t + 1) * P]
                    .rearrange("(a p) n -> p a n", p=P), ts[:])

            def make_epi(bias_idx, dramT):
                def epi(t, mps):
                    o = sb.tile([P, FD], BF16, tag="oed")
                    _norm_elu(mps, btiles[bias_idx], o)
                    _tstore(o, dramT, t)
                return epi

            # ---- L1 edge phases
            edge_phase('dd',
                       [tabDD.opt()[0:HALF, :], tabDD.opt()[HALF:NDP, :]],
                       lambda t: erDDs[:, t * 8:(t + 1) * 8],
                       sched['dd'], make_epi(0, hd1T.opt()))
            nc.gpsimd.collective_compute(
                "AllGather", AluOp.bypass,
                replica_groups=[list(range(nco))],
                ins=[hd1T.opt()], outs=[hd1T_ag.opt()])
            edge_phase('dc',
                       [tabDC.opt()[0:HALF, :], tabDC.opt()[HALF:NDP, :]],
                       lambda t: erC1s[:, t * 16:t * 16 + 8],
                       sched['dc'], make_epi(1, oDC1T.opt()))
            edge_phase('cc', [tabCC1.opt()[:, :]],
                       lambda t: erC1s[:, t * 16 + 8:t * 16 + 16],
                       sched['cc'], make_epi(2, oCC1T.opt()))

            # ---- semantic attention (cells) + optional er stash / head
            def sem_tile(l, oDCT, oCCT, t, consume):
                z = []
                for srcT in (oDCT, oCCT):
                    zt = sb.tile([P, 2, P], BF16, tag="semz")
                    nc.sync.dma_start(
                        zt[:], srcT[:, t * P:(t + 1) * P]
                        .rearrange("(a p) n -> p a n", p=P))
                    z.append(zt)
                wms = []
                for m in range(2):
                    hp = ps.tile([P, P], F32, tag="aux")
                    nc.tensor.matmul(hp[:], lhsT=sw1[l][0][:], rhs=z[m][:, 0, :],
                                     start=True, stop=False)
                    nc.tensor.matmul(hp[:], lhsT=sw1[l][1][:], rhs=z[m][:, 1, :],
                                     start=False, stop=True)
                    ht = sb.tile([P, P], BF16, tag="semh")
                    nc.scalar.activation(ht[:], hp[:], Act.Tanh, bias=sb1[l][:])
                    wp = ps.tile([1, P], F32, tag="aux")
                    nc.tensor.matmul(wp[:], lhsT=sw2[l][:], rhs=ht[:],
                                     start=True, stop=True)
                    wm = sb.tile([1, P], F32, tag="semw")
                    nc.scalar.copy(wm[:], wp[:])
                    wms.append(wm)
                beta = sb.tile([1, P], BF16, tag="semb")
                nc.vector.tensor_sub(beta[:], wms[0][:], wms[1][:])
                nc.scalar.activation(beta[:], beta[:], Act.Sigmoid)
                bb = ps.tile([P, P], F32, tag="aux")
                nc.tensor.matmul(bb[:], lhsT=ones1[:], rhs=beta[:],
                                 start=True, stop=True)
                hcts = []
                for kk in range(2):
                    diff = sb.tile([P, P], BF16, tag="semd")
                    nc.vector.tensor_sub(diff[:], z[0][:, kk, :], z[1][:, kk, :])
                    nc.vector.tensor_mul(diff[:], diff[:], bb[:])
                    hct = sb.tile([P, P], BF16, tag="semhc")
                    nc.vector.tensor_add(hct[:], z[1][:, kk, :], diff[:])
                    hcts.append(hct)
                consume(t, hcts)

            def sem1_consume(t, hcts):
                ht = sb.tile([P, 2, P], BF16, tag="hc1w")
                nc.vector.tensor_copy(ht[:, 0, :], hcts[0][:])
                nc.vector.tensor_copy(ht[:, 1, :], hcts[1][:])
                nc.sync.dma_start(
                    hc1T.opt()[:, t * P:(t + 1) * P]
                    .rearrange("(a p) n -> p a n", p=P), ht[:])
                ep = ps.tile([P, 16], F32, tag="aux")
                nc.tensor.matmul(ep[:], lhsT=hcts[0][:], rhs=werC2[0][:],
                                 start=True, stop=False)
                nc.tensor.matmul(ep[:], lhsT=hcts[1][:], rhs=werC2[1][:],
                                 start=False, stop=True)
                nc.scalar.copy(erC2s[:, t * 16:(t + 1) * 16], ep[:])

            for t in range(CT):
                sem_tile(0, oDC1T.opt(), oCC1T.opt(), t, sem1_consume)
            nc.gpsimd.collective_compute(
                "AllGather", AluOp.bypass,
                replica_groups=[list(range(nco))],
                ins=[hc1T.opt()], outs=[hc1T_ag.opt()])

            # ---- L2 projections from AllGathered features
            agD = hd1T_ag.opt().rearrange("(c a p) n -> c a p n", a=2, p=P)
            for c in range(nco):
                proj(lambda k, c0, n, c=c: agD[c, k, :, c0:c0 + n], DD_T,
                     [(wt['WDC2'], tabDC2.opt()[c * DBLK:(c + 1) * DBLK, :])])
            agC = hc1T_ag.opt().rearrange("(c a p) n -> c a p n", a=2, p=P)
            for c in range(nco):
                proj(lambda k, c0, n, c=c: agC[c, k, :, c0:c0 + n], CT,
                     [(wt['WCC2'], tabCC2.opt()[c * CBLK:(c + 1) * CBLK, :])],
                     BT=CT)

            # ---- L2 edge phases
            edge_phase('dc',
                       [tabDC2.opt()[0:HALF, :], tabDC2.opt()[HALF:NDP, :]],
                       lambda t: erC2s[:, t * 16:t * 16 + 8],
                       sched['dc'], make_epi(3, oDC2T.opt()))
            edge_phase('cc', [tabCC2.opt()[:, :]],
                       lambda t: erC2s[:, t * 16 + 8:t * 16 + 16],
                       sched['cc'], make_epi(4, oCC2T.opt()))

            # ---- sem2 + MLP head
            def sem2_consume(t, hcts):
                h1p = ps.tile([32, P], F32, tag="aux")
                nc.tensor.matmul(h1p[:], lhsT=dW1[0][:], rhs=hcts[0][:],
                                 start=True, stop=False)
                nc.tensor.matmul(h1p[:], lhsT=dW1[1][:], rhs=hcts[1][:],
                                 start=False, stop=True)
                h1 = sb.tile([32, P], BF16, tag="mlph1")
                nc.vector.scalar_tensor_tensor(
                    out=h1[:], in0=h1p[:], scalar=1.0,
                    in1=db1[:].to_broadcast([32, P]),
                    op0=AluOp.mult, op1=AluOp.add)
                nc.vector.scalar_tensor_tensor(
                    out=h1[:], in0=h1[:], scalar=0.01, in1=h1[:],
                    op0=AluOp.mult, op1=AluOp.max)
                h2p = ps.tile([16, P], F32, tag="aux")
                nc.tensor.matmul(h2p[:], lhsT=dW2[:], rhs=h1[:],
                                 start=True, stop=True)
                h2 = sb.tile([16, P], BF16, tag="mlph2")
                nc.vector.scalar_tensor_tensor(
                    out=h2[:], in0=h2p[:], scalar=1.0,
                    in1=db2[:].to_broadcast([16, P]),
                    op0=AluOp.mult, op1=AluOp.add)
                nc.vector.scalar_tensor_tensor(
                    out=h2[:], in0=h2[:], scalar=0.01, in1=h2[:],
                    op0=AluOp.mult, op1=AluOp.max)
                h3p = ps.tile([1, P], F32, tag="aux")
                nc.tensor.matmul(h3p[:], lhsT=dW3[:], rhs=h2[:],
                                 start=True, stop=True)
                h3 = sb.tile([1, P], F32, tag="mlph3")
                nc.vector.tensor_scalar(h3[:], h3p[:], db3[:], None,
                                        op0=AluOp.add)
                nc.sync.dma_start(out[0:1, t * P:(t + 1) * P], h3[:])

            for t in range(CT):
                sem_tile(1, oDC2T.opt(), oCC2T.opt(), t, sem2_consume)

    nc.compile()
    if legalize:
        legalize_waits(nc)
    return nc


# --------------------------------------------------------------------------
# entry point
# --------------------------------------------------------------------------

_CACHE = {}


def kernel(**inputs):
    cfg = make_cfg(inputs['feat_drug'].shape[0], inputs['feat_cell'].shape[0])
    sched, in_maps = host_prep(inputs, cfg)
    key = tuple(int(x) for s in sched.values() for x in s.flatten())
    if key not in _CACHE:
        _CACHE[key] = build_program(sched, cfg)
    nc = _CACHE[key]
    from concourse.bass_utils import run_bass_kernel_spmd
    res = run_bass_kernel_spmd(nc, in_maps, list(range(cfg['n_cores'])))
    pieces = [res.results[c]['out'][0] for c in range(cfg['n_cores'])]
    full = np.concatenate([p[:cfg['CBLK']] for p in pieces])[:cfg['Nc']]
    return full.reshape(-1, 1).astype(np.float32)


# revision 17
# speedup vs baseline: 2.1525x; 1.6082x over previous
"""Trainium2 Bass kernel v2 for the 2-layer heterogeneous GAT (drug/cell).

Strategy (8 NeuronCores, SPMD single program), changes vs v1:
  - All five edge phases partitioned by DST block; dc2 now dst-partitioned
    too, fed by an AllGather of hd1 (bf16) instead of AllToAll partials.
  - bf16 gather tables, 384-col rows (768B, %256B) holding [fs 256 | el 8].
  - er is never gathered: per-dst-tile er vectors [128, 8] are stashed in
    SBUF (projected from local features / hc1 tiles) and broadcast to edges
    with a one-hot matmul (lhsT = S2[dst -> edge]).
  - Edge phase batches DVE work per group of <=16 chunks: one is_equal for
    all S chunks, one for S2 (dst-row stream replicated via DMA broadcast),
    one fused exp/leaky pipeline, one rhs build.
  - Gathers up to 1024 idx per call (SWDGE ring limit is < 2048).
  - Projections batched: 8 tiles per DMA load/store, psum->SBUF copies
    alternate ACT/DVE.
"""
import sys
sys.path.insert(0, '/opt/trn_rl_repo')
import numpy as np
import ml_dtypes

import concourse.bacc as bacc
import concourse.tile as tile
from concourse import mybir, library_config

F32 = mybir.dt.float32
BF16 = mybir.dt.bfloat16
FP8 = mybir.dt.float8e4
I16 = mybir.dt.int16
BF = ml_dtypes.bfloat16
F8 = mybir.dt.np(FP8)
P = 128
H = 8
FD = 256          # feature dim
TW = 384          # gather-table row width (bf16) = 768B; payload 264
GSZ = 16          # chunks per batched edge group
GCH = 8           # chunks per dma_gather call (1024 idx; 2048 hangs)
AluOp = mybir.AluOpType
Act = mybir.ActivationFunctionType

N_CORES = 8


def legalize_waits(nc):
    """Split multi-wait instructions into wait-carrying NOP chains."""
    n_split = 0
    for fn in nc.m.functions:
        for bb in fn.blocks:
            insts = bb.instructions
            new = []
            changed = False
            for inst in insts:
                si = inst.sync_info
                waits = list(si.on_wait) if si is not None else []
                cap = 2 if isinstance(inst, mybir.InstEventSemaphore) else 1
                if len(waits) > cap:
                    keep = waits[-cap:]
                    for w in waits[:-cap]:
                        nop = mybir.InstNoOp(
                            name=nc.get_next_instruction_name(),
                            engine=inst.engine,
                            sync_info=mybir.SyncInfo(on_wait=[w], on_update=[]),
                            bass_nofuse=True,
                        )
                        new.append(nop)
                        n_split += 1
                    inst.sync_info = mybir.SyncInfo(
                        on_wait=keep, on_update=list(si.on_update))
                    changed = True
                new.append(inst)
            if changed:
                bb.instructions = new
    return n_split


def make_cfg(Nd, Nc):
    ndp = -(-Nd // 1024) * 1024
    ncp = -(-Nc // 1024) * 1024
    return dict(Nd=Nd, Nc=Nc, NDP=ndp, NCP=ncp, DBLK=ndp // 8,
                CBLK=ncp // 8, HALF=ndp // 2, n_cores=8)


# --------------------------------------------------------------------------
# host-side prep
# --------------------------------------------------------------------------

def _fold_weights(ip):
    def wel(W, a):  # W [256,256], a [H,D] -> [256,H]
        return (np.asarray(W, np.float32).reshape(FD, H, -1)
                * np.asarray(a, np.float32)[None]).sum(-1)
    Wsrc = np.asarray(ip['Wsrc'], np.float32)
    Wdst = np.asarray(ip['Wdst_dc'], np.float32)
    al = np.asarray(ip['attn_l'], np.float32)
    ar = np.asarray(ip['attn_r'], np.float32)

    def w264(W, a):
        return np.concatenate([W, wel(W, a)], 1)  # [256, 264]

    def bftile(a, ncols):  # [256, ncols] -> [2, 128, ncols] bf16
        return np.ascontiguousarray(a.reshape(2, P, ncols)).astype(BF)

    out = {}
    out['WDD'] = bftile(w264(Wsrc[0, 0], al[0, 0]), 264)
    out['WDC'] = bftile(w264(Wsrc[0, 1], al[0, 1]), 264)
    out['WCC1'] = bftile(w264(Wsrc[0, 2], al[0, 2]), 264)
    out['WDC2'] = bftile(w264(Wsrc[1, 1], al[1, 1]), 264)
    out['WCC2'] = bftile(w264(Wsrc[1, 2], al[1, 2]), 264)
    out['WerDD'] = bftile(wel(Wsrc[0, 0], ar[0, 0]), 8)
    out['WerC1'] = bftile(np.concatenate(
        [wel(Wdst[0], ar[0, 1]), wel(Wsrc[0, 2], ar[0, 2])], 1), 16)
    out['WerC2'] = bftile(np.concatenate(
        [wel(Wdst[1], ar[1, 1]), wel(Wsrc[1, 2], ar[1, 2])], 1), 16)
    out['bias5'] = np.stack([ip['gat_bias'][0, 0], ip['gat_bias'][0, 1],
                             ip['gat_bias'][0, 2], ip['gat_bias'][1, 1],
                             ip['gat_bias'][1, 2]]).astype(np.float32)
    out['semW1'] = np.stack([np.asarray(ip['sem_W1'][l, 1], np.float32)
                             .reshape(2, P, P) for l in range(2)]).astype(BF)
    out['semb1'] = np.stack([np.asarray(ip['sem_b1'][l, 1], np.float32)
                             .reshape(P, 1) for l in range(2)]).astype(np.float32)
    out['semW2'] = np.stack([np.asarray(ip['sem_W2'][l, 1], np.float32)
                             .reshape(P, 1) for l in range(2)]).astype(BF)
    out['dnnW1'] = np.asarray(ip['dnn_W1'], np.float32).reshape(2, P, 32).astype(BF)
    out['dnnb1'] = np.asarray(ip['dnn_b1'], np.float32).reshape(32, 1)
    out['dnnW2'] = np.asarray(ip['dnn_W2'], np.float32).astype(BF)
    out['dnnb2'] = np.asarray(ip['dnn_b2'], np.float32).reshape(16, 1)
    out['dnnW3'] = np.asarray(ip['dnn_W3'], np.float32).astype(BF)
    out['dnnb3'] = np.asarray(ip['dnn_b3'], np.float32).reshape(1, 1)
    return out


def _wrap16(vals):
    n = len(vals)
    out = np.zeros((P, n // 16), np.int16)
    a = np.asarray(vals, np.int16).reshape(-1, 16).T
    for g in range(8):
        out[g * 16:(g + 1) * 16, :] = a
    return out


def _prep_rel(src, dst, nco, *, blk, n_tiles, halves):
    """Per-core edge schedule, dst-partitioned.

    Returns nch [T, n_halves] (equalized over cores) and per-core streams:
    src idx wrapped i16 [P, tot*8], plus host-built one-hot scatter
    matrices in fp8: S [P, tot*128] (S[p, k*128+d] = dl[p,k]==d) and
    S2 [P, tot*128] (S2[p, k*128+e] = dl[e,k]==p).
    """
    src = np.asarray(src, np.int64)
    dst = np.asarray(dst, np.int64)
    per_core = []
    for c in range(nco):
        m = (dst >= c * blk) & (dst < (c + 1) * blk)
        dl = dst[m] - c * blk
        s = src[m]
        tid = dl // P
        per_core.append((s, dl % P, tid))
    nh = 2 if halves else 1
    nch = np.zeros((n_tiles, nh), np.int64)
    buckets = []
    for c in range(nco):
        s, dloc, tid = per_core[c]
        bk = {}
        for t in range(n_tiles):
            mt = tid == t
            st, dt_ = s[mt], dloc[mt]
            if halves:
                m0 = st < halves
                groups = [(st[m0], dt_[m0]), (st[~m0] - halves, dt_[~m0])]
            else:
                groups = [(st, dt_)]
            bk[t] = groups
            for h, (gs, gd) in enumerate(groups):
                nch[t, h] = max(nch[t, h], (len(gs) + P - 1) // P)
        buckets.append(bk)
    nch = np.maximum(nch, 1)
    tot = int(nch.sum())
    rng = np.arange(P, dtype=np.int64)
    srcs, Ss, S2s = [], [], []
    for c in range(nco):
        bk = buckets[c]
        sw = np.zeros((P, tot * 8), np.int16)
        dall = np.full((tot, P), -1, np.int64)   # [chunk, edge] local dst
        col = 0
        for t in range(n_tiles):
            for h, (gs, gd) in enumerate(bk[t]):
                n = int(nch[t, h]) * P
                a = np.zeros(n, np.int64)
                a[:len(gs)] = gs
                assert a.max(initial=0) < 32768
                sw[:, col * 8:(col + int(nch[t, h])) * 8] = _wrap16(a)
                dpad = np.full(n, -1, np.int64)
                dpad[:len(gd)] = gd
                dall[col:col + int(nch[t, h])] = dpad.reshape(-1, P)
                col += int(nch[t, h])
        # S[p, k, d] = (dall[k, p] == d);  S2[p, k, e] = (dall[k, e] == p)
        S = (dall.T[:, :, None] == rng[None, None, :]).astype(F8)
        S2 = (dall[None, :, :] == rng[:, None, None]).astype(F8)
        srcs.append(sw)
        Ss.append(np.ascontiguousarray(S.reshape(P, tot * P)))
        S2s.append(np.ascontiguousarray(S2.reshape(P, tot * P)))
    return dict(nch=nch, src=srcs, S=Ss, S2=S2s, tot=tot)


def host_prep(ip, cfg):
    W = _fold_weights(ip)
    nco = cfg['n_cores']
    DBLK, CBLK, HALF = cfg['DBLK'], cfg['CBLK'], cfg['HALF']
    NDP, NCP = cfg['NDP'], cfg['NCP']
    DD_T, CT = DBLK // P, CBLK // P

    dd = _prep_rel(ip['src_dd'], ip['dst_dd'], nco, blk=DBLK,
                   n_tiles=DD_T, halves=HALF)
    dc = _prep_rel(ip['src_dc'], ip['dst_dc'], nco, blk=CBLK,
                   n_tiles=CT, halves=HALF)
    cc = _prep_rel(ip['src_cc'], ip['dst_cc'], nco, blk=CBLK,
                   n_tiles=CT, halves=None)

    featD = np.zeros((NDP, FD), np.float32)
    featD[:cfg['Nd']] = np.asarray(ip['feat_drug'], np.float32)
    featC = np.zeros((NCP, FD), np.float32)
    featC[:cfg['Nc']] = np.asarray(ip['feat_cell'], np.float32)
    featDT = np.ascontiguousarray(featD.T).reshape(2, P, NDP).astype(BF)
    featCT = np.ascontiguousarray(featC.T).reshape(2, P, NCP).astype(BF)

    identB = np.eye(P, dtype=np.float32).astype(BF)

    base = dict(featDT=featDT, featCT=featCT, identB=identB, **W)
    in_maps = []
    for c in range(nco):
        m = dict(base)
        m['featDTloc'] = np.ascontiguousarray(
            featD[c * DBLK:(c + 1) * DBLK].T).reshape(2, P, DBLK).astype(BF)
        m['featCTloc'] = np.ascontiguousarray(
            featC[c * CBLK:(c + 1) * CBLK].T).reshape(2, P, CBLK).astype(BF)
        for r, d in (('dd', dd), ('dc', dc), ('cc', cc)):
            m[f'{r}_src'] = d['src'][c]
            m[f'{r}_S'] = d['S'][c]
            m[f'{r}_S2'] = d['S2'][c]
        in_maps.append(m)
    sched = dict(dd=dd['nch'], dc=dc['nch'], cc=cc['nch'])
    return sched, in_maps


# --------------------------------------------------------------------------
# device program
# --------------------------------------------------------------------------

def _tile_segments(nch_row):
    """Per-tile gather segments [(start_col, n, half)], respecting half
    boundaries, GSZ group windows, and GCH call caps."""
    bounds = []
    off = 0
    for h, nh in enumerate(nch_row):
        bounds.append((off, off + int(nh), h))
        off += int(nh)
    tnch = off
    segs = []
    for g0 in range(0, tnch, GSZ):
        g1 = min(g0 + GSZ, tnch)
        for (h0, h1, h) in bounds:
            s0 = max(g0, h0)
            s1 = min(g1, h1)
            for b in range(s0, s1, GCH):
                segs.append((b, min(GCH, s1 - b), h))
    return segs, tnch


def build_program(sched, cfg, legalize=True):
    nco = cfg['n_cores']
    DBLK, CBLK, HALF = cfg['DBLK'], cfg['CBLK'], cfg['HALF']
    NDP, NCP = cfg['NDP'], cfg['NCP']
    DD_T, CT = DBLK // P, CBLK // P
    NDT, NCT = NDP // P, NCP // P

    nc = bacc.Bacc(None)
    d = {}
    def inp(name, shape, dt=BF16):
        d[name] = nc.declare_dram_parameter(name, list(shape), dt,
                                            isOutput=False)
        return d[name]

    featDT = inp('featDT', (2, P, NDP))
    featCT = inp('featCT', (2, P, NCP))
    featDTloc = inp('featDTloc', (2, P, DBLK))
    featCTloc = inp('featCTloc', (2, P, CBLK))
    identB_in = inp('identB', (P, P))
    Wmain = {k: inp(k, (2, P, 264)) for k in
             ('WDD', 'WDC', 'WCC1', 'WDC2', 'WCC2')}
    WerDD = inp('WerDD', (2, P, 8))
    WerC1 = inp('WerC1', (2, P, 16))
    WerC2 = inp('WerC2', (2, P, 16))
    bias5 = inp('bias5', (5, FD), F32)
    semW1 = inp('semW1', (2, 2, P, P))
    semb1 = inp('semb1', (2, P, 1), F32)
    semW2 = inp('semW2', (2, P, 1))
    dnnW1 = inp('dnnW1', (2, P, 32)); dnnb1 = inp('dnnb1', (32, 1), F32)
    dnnW2 = inp('dnnW2', (32, 16)); dnnb2 = inp('dnnb2', (16, 1), F32)
    dnnW3 = inp('dnnW3', (16, 1)); dnnb3 = inp('dnnb3', (1, 1), F32)
    streams = {}
    for r, nchs in sched.items():
        tot = int(nchs.sum())
        streams[r] = (inp(f'{r}_src', (P, tot * 8), I16),
                      inp(f'{r}_S', (P, tot * P), FP8),
                      inp(f'{r}_S2', (P, tot * P), FP8))
    out = nc.declare_dram_parameter('out', [1, CBLK], F32, isOutput=True)

    with tile.TileContext(nc) as tc:
        with tc.tile_pool(name="const", bufs=1) as cpool, \
             tc.tile_pool(name="sb", bufs=3) as sb, \
             tc.tile_pool(name="pl", bufs=3) as pl, \
             tc.tile_pool(name="gb", bufs=3) as gb, \
             tc.tile_pool(name="eb", bufs=3) as eb, \
             tc.tile_pool(name="ps", bufs=2, space="PSUM") as ps, \
             tc.tile_pool(name="psm", bufs=2, space="PSUM") as psm, \
             tc.tile_pool(name="dram", bufs=1, space="DRAM") as dr:
            nc.gpsimd.load_library(library_config.mlp)

            # ---- constants in SBUF
            C = {}
            def cload(name, ap, shape, dt=BF16):
                t_ = cpool.tile(list(shape), dt, tag=f"c_{name}")
                nc.sync.dma_start(t_[:], ap)
                return t_
            C['ident'] = cload('ident', identB_in[:], (P, P))
            ones1 = cpool.tile([1, P], BF16)
            nc.vector.memset(ones1[:], 1.0)
            wt = {}
            for k in Wmain:
                wt[k] = tuple(cload(f'{k}{j}', Wmain[k][j], (P, 264))
                              for j in range(2))
            werDD = tuple(cload(f'werDD{j}', WerDD[j], (P, 8))
                          for j in range(2))
            werC1 = tuple(cload(f'werC1{j}', WerC1[j], (P, 16))
                          for j in range(2))
            werC2 = tuple(cload(f'werC2{j}', WerC2[j], (P, 16))
                          for j in range(2))
            btiles = [cload(f'bias{r}', bias5[r:r + 1, :].to_broadcast([P, FD]),
                            (P, FD), F32) for r in range(5)]
            sw1 = [tuple(cload(f'sw1_{l}{j}', semW1[l, j], (P, P))
                         for j in range(2)) for l in range(2)]
            sb1 = [cload(f'sb1_{l}', semb1[l], (P, 1), F32) for l in range(2)]
            sw2 = [cload(f'sw2_{l}', semW2[l], (P, 1)) for l in range(2)]
            dW1 = tuple(cload(f'dW1{j}', dnnW1[j], (P, 32)) for j in range(2))
            dW2 = cload('dW2', dnnW2[:], (32, 16))
            dW3 = cload('dW3', dnnW3[:], (16, 1))
            db1 = cload('db1', dnnb1[:], (32, 1), F32)
            db2 = cload('db2', dnnb2[:], (16, 1), F32)
            db3 = cload('db3', dnnb3[:], (1, 1), F32)
            # resident local features (for er projections)
            fdl = tuple(cload(f'fdl{j}', featDTloc[j], (P, DBLK))
                        for j in range(2))
            fcl = tuple(cload(f'fcl{j}', featCTloc[j], (P, CBLK))
                        for j in range(2))
            # resident edge streams
            SI = {}
            for r in sched:
                tot = int(sched[r].sum())
                SI[r] = cload(f'si_{r}', streams[r][0][:], (P, tot * 8), I16)
            # er stashes
            erDDs = cpool.tile([P, DD_T * 8], BF16, tag="erDDs")
            erC1s = cpool.tile([P, CT * 16], BF16, tag="erC1s")
            erC2s = cpool.tile([P, CT * 16], BF16, tag="erC2s")

            # ---- internal DRAM
            tabDD = dr.tile([NDP, TW], BF16)
            tabDC = dr.tile([NDP, TW], BF16)
            tabCC1 = dr.tile([NCP, TW], BF16)
            tabDC2loc = dr.tile([DBLK, TW], BF16)
            tabDC2 = dr.tile([NDP, TW], BF16, addr_space="Shared")
            tabCC2loc = dr.tile([CBLK, TW], BF16)
            tabCC2 = dr.tile([NCP, TW], BF16, addr_space="Shared")
            hd1T = dr.tile([2 * P, DBLK], BF16)
            oDC1T = dr.tile([2 * P, CBLK], BF16)
            oCC1T = dr.tile([2 * P, CBLK], BF16)
            oDC2T = dr.tile([2 * P, CBLK], BF16)
            oCC2T = dr.tile([2 * P, CBLK], BF16)
            hc1T = dr.tile([2 * P, CBLK], BF16)

            # ---- er stash projections (local features)
            def er_stash(fres, wpair, stash, n_tiles, ncols):
                BT = min(8, n_tiles) if ncols == 8 else n_tiles
                for t0 in range(0, n_tiles, BT):
                    bt = min(BT, n_tiles - t0)
                    pp = ps.tile([P, bt * ncols], F32, tag="projps")
                    for i in range(bt):
                        tl = (t0 + i) * P
                        nc.tensor.matmul(pp[:, i * ncols:(i + 1) * ncols],
                                         lhsT=fres[0][:, tl:tl + P],
                                         rhs=wpair[0][:], start=True, stop=False)
                        nc.tensor.matmul(pp[:, i * ncols:(i + 1) * ncols],
                                         lhsT=fres[1][:, tl:tl + P],
                                         rhs=wpair[1][:], start=False, stop=True)
                    nc.scalar.copy(stash[:, t0 * ncols:(t0 + bt) * ncols], pp[:])
            er_stash(fdl, werDD, erDDs, DD_T, 8)
            er_stash(fcl, werC1, erC1s, CT, 16)

            # ---- batched projection pass (multi-job: shared lhs loads)
            def proj(lhs_ap_fn, n_tiles, jobs, BT=8):
                """lhs_ap_fn(k, c0, n) -> DRAM AP [P, n] for k-chunk cols.
                jobs: list of (wpair, tab)."""
                for t0 in range(0, n_tiles, BT):
                    bt = min(BT, n_tiles - t0)
                    lh = pl.tile([P, 2, bt * P], BF16, tag="projlh")
                    nc.sync.dma_start(lh[:, 0, :], lhs_ap_fn(0, t0 * P, bt * P))
                    nc.sync.dma_start(lh[:, 1, :], lhs_ap_fn(1, t0 * P, bt * P))
                    for j, (wpair, tab) in enumerate(jobs):
                        ob = pl.tile([P, bt, 264], BF16, tag=f"projout{j}")
                        for i in range(bt):
                            pp = ps.tile([P, 264], F32, tag="projps")
                            nc.tensor.matmul(pp[:],
                                             lhsT=lh[:, 0, i * P:(i + 1) * P],
                                             rhs=wpair[0][:],
                                             start=True, stop=False)
                            nc.tensor.matmul(pp[:],
                                             lhsT=lh[:, 1, i * P:(i + 1) * P],
                                             rhs=wpair[1][:],
                                             start=False, stop=True)
                            nc.scalar.copy(ob[:, i, :], pp[:])
                        nc.sync.dma_start(
                            tab[t0 * P:(t0 + bt) * P, 0:264]
                            .rearrange("(t p) c -> p t c", p=P), ob[:])

            def dram_lhs(apx):
                return lambda k, c0, n: apx[k, :, c0:c0 + n]

            # L1 projections: tabDD+tabDC share feature loads
            proj(dram_lhs(featDT), NDT, [(wt['WDD'], tabDD.opt()),
                                         (wt['WDC'], tabDC.opt())])
            proj(dram_lhs(featCT), NCT, [(wt['WCC1'], tabCC1.opt())])

            # ---- edge phase
            def edge_phase(rel, tab_halves, er_fn, nchs, epilogue):
                si = SI[rel]
                S_dram, S2_dram = streams[rel][1], streams[rel][2]
                T = nchs.shape[0]
                col = 0
                for t in range(T):
                    segs, tnch = _tile_segments(nchs[t])
                    mps = psm.tile([P, 264], F32, tag="mainps")
                    er_t = er_fn(t)
                    for g0 in range(0, tnch, GSZ):
                        gn = min(GSZ, tnch - g0)
                        G = gb.tile([P, gn, TW], BF16, tag="G")
                        for (s0, sn, h) in segs:
                            if s0 < g0 or s0 >= g0 + gn:
                                continue
                            nidx = sn * P
                            nc.gpsimd.dma_gather(
                                G[:, s0 - g0:s0 - g0 + sn, :], tab_halves[h],
                                si[:, (col + s0) * 8:(col + s0 + sn) * 8],
                                nidx, nidx, TW)
                        S = eb.tile([P, gn * P], FP8, tag="S")
                        nc.sync.dma_start(
                            S[:], S_dram[:, (col + g0) * P:(col + g0 + gn) * P])
                        S2 = eb.tile([P, gn * P], FP8, tag="S2")
                        nc.sync.dma_start(
                            S2[:], S2_dram[:, (col + g0) * P:(col + g0 + gn) * P])
                        erps = ps.tile([P, gn * 8], F32, tag="erps")
                        for k in range(gn):
                            nc.tensor.matmul(
                                erps[:, k * 8:(k + 1) * 8],
                                lhsT=S2[:, k * P:(k + 1) * P], rhs=er_t,
                                start=True, stop=True, skip_group_check=True)
                        ww = eb.tile([P, gn, 8], F32, tag="ww")
                        nc.vector.tensor_tensor(
                            out=ww[:], in0=G[:, :, FD:FD + 8],
                            in1=erps[:].rearrange("p (a b) -> p a b", b=8),
                            op=AluOp.add)
                        nc.vector.scalar_tensor_tensor(
                            out=ww[:], in0=ww[:], scalar=0.2, in1=ww[:],
                            op0=AluOp.mult, op1=AluOp.max)
                        nc.scalar.activation(ww[:], ww[:], Act.Exp)
                        rhs = eb.tile([P, gn, 264], BF16, tag="rhs")
                        nc.vector.tensor_tensor(
                            out=rhs[:, :, 0:FD]
                                .rearrange("p a (h e) -> p a h e", h=H),
                            in0=G[:, :, 0:FD]
                                .rearrange("p a (h e) -> p a h e", h=H),
                            in1=ww[:, :, :, None].to_broadcast([P, gn, H, 32]),
                            op=AluOp.mult)
                        nc.vector.tensor_copy(rhs[:, :, FD:FD + 8], ww[:])
                        for k in range(gn):
                            nc.tensor.matmul(
                                mps[:], lhsT=S[:, k * P:(k + 1) * P],
                                rhs=rhs[:, k, :],
                                start=(g0 == 0 and k == 0),
                                stop=(g0 + gn == tnch and k == gn - 1),
                                skip_group_check=True)
                    epilogue(t, mps)
                    col += tnch

            def _norm_elu(pp, bias_tile, o_bf):
                den = sb.tile([P, 8], F32, tag="den")
                nc.vector.tensor_scalar_max(den[:], pp[:, FD:FD + 8], 1e-30)
                rec = sb.tile([P, 8], F32, tag="rec")
                nc.vector.reciprocal(rec[:], den[:])
                x = sb.tile([P, FD], F32, tag="xnrm")
                nc.vector.tensor_tensor(
                    out=x[:].rearrange("p (h e) -> p h e", h=H),
                    in0=pp[:, 0:FD].rearrange("p (h e) -> p h e", h=H),
                    in1=rec[:, :, None].to_broadcast([P, H, 32]),
                    op=AluOp.mult)
                nc.vector.tensor_add(x[:], x[:], bias_tile[:])
                ex = sb.tile([P, FD], F32, tag="eluex")
                nc.scalar.activation(ex[:], x[:], Act.Exp)
                nc.scalar.activation(ex[:], ex[:], Act.Relu, bias=1.0, scale=-1.0)
                xp = sb.tile([P, FD], F32, tag="elup")
                nc.scalar.activation(xp[:], x[:], Act.Relu)
                nc.vector.tensor_sub(o_bf[:], xp[:], ex[:])

            def _tstore(o_bf, dramT, t):
                ts = sb.tile([P, 2, P], BF16, tag="tpsb")
                for kk in range(2):
                    tp = ps.tile([P, P], BF16, space="PSUM", tag="aux")
                    nc.tensor.transpose(tp[:], o_bf[:, kk * P:(kk + 1) * P],
                                        C['ident'][:])
                    nc.scalar.copy(ts[:, kk, :], tp[:])
                nc.sync.dma_start(
                    dramT[:, t * P:(t + 1) * P]
                    .rearrange("(a p) n -> p a n", p=P), ts[:])

            def make_epi(bias_idx, dramT):
                def epi(t, mps):
                    o = sb.tile([P, FD], BF16, tag="oed")
                    _norm_elu(mps, btiles[bias_idx], o)
                    _tstore(o, dramT, t)
                return epi

            # ---- L1 edge phases
            edge_phase('dd',
                       [tabDD.opt()[0:HALF, :], tabDD.opt()[HALF:NDP, :]],
                       lambda t: erDDs[:, t * 8:(t + 1) * 8],
                       sched['dd'], make_epi(0, hd1T.opt()))
            # local tabDC2 shard from local hd1, then AllGather the table
            proj(dram_lhs(hd1T.opt().rearrange("(a p) n -> a p n", p=P)),
                 DD_T, [(wt['WDC2'], tabDC2loc.opt())])
            nc.gpsimd.collective_compute(
                "AllGather", AluOp.bypass,
                replica_groups=[list(range(nco))],
                ins=[tabDC2loc.opt()], outs=[tabDC2.opt()])
            edge_phase('dc',
                       [tabDC.opt()[0:HALF, :], tabDC.opt()[HALF:NDP, :]],
                       lambda t: erC1s[:, t * 16:t * 16 + 8],
                       sched['dc'], make_epi(1, oDC1T.opt()))
            edge_phase('cc', [tabCC1.opt()[:, :]],
                       lambda t: erC1s[:, t * 16 + 8:t * 16 + 16],
                       sched['cc'], make_epi(2, oCC1T.opt()))

            # ---- semantic attention (cells) + optional er stash / head
            def sem_tile(l, oDCT, oCCT, t, consume):
                z = []
                for srcT in (oDCT, oCCT):
                    zt = sb.tile([P, 2, P], BF16, tag="semz")
                    nc.sync.dma_start(
                        zt[:], srcT[:, t * P:(t + 1) * P]
                        .rearrange("(a p) n -> p a n", p=P))
                    z.append(zt)
                wms = []
                for m in range(2):
                    hp = ps.tile([P, P], F32, tag="aux")
                    nc.tensor.matmul(hp[:], lhsT=sw1[l][0][:], rhs=z[m][:, 0, :],
                                     start=True, stop=False)
                    nc.tensor.matmul(hp[:], lhsT=sw1[l][1][:], rhs=z[m][:, 1, :],
                                     start=False, stop=True)
                    ht = sb.tile([P, P], BF16, tag="semh")
                    nc.scalar.activation(ht[:], hp[:], Act.Tanh, bias=sb1[l][:])
                    wp = ps.tile([1, P], F32, tag="aux")
                    nc.tensor.matmul(wp[:], lhsT=sw2[l][:], rhs=ht[:],
                                     start=True, stop=True)
                    wm = sb.tile([1, P], F32, tag="semw")
                    nc.scalar.copy(wm[:], wp[:])
                    wms.append(wm)
                beta = sb.tile([1, P], BF16, tag="semb")
                nc.vector.tensor_sub(beta[:], wms[0][:], wms[1][:])
                nc.scalar.activation(beta[:], beta[:], Act.Sigmoid)
                bb = ps.tile([P, P], F32, tag="aux")
                nc.tensor.matmul(bb[:], lhsT=ones1[:], rhs=beta[:],
                                 start=True, stop=True)
                hcts = []
                for kk in range(2):
                    diff = sb.tile([P, P], BF16, tag="semd")
                    nc.vector.tensor_sub(diff[:], z[0][:, kk, :], z[1][:, kk, :])
                    nc.vector.tensor_mul(diff[:], diff[:], bb[:])
                    hct = sb.tile([P, P], BF16, tag="semhc")
                    nc.vector.tensor_add(hct[:], z[1][:, kk, :], diff[:])
                    hcts.append(hct)
                consume(t, hcts)

            def sem1_consume(t, hcts):
                ht = sb.tile([P, 2, P], BF16, tag="hc1w")
                nc.vector.tensor_copy(ht[:, 0, :], hcts[0][:])
                nc.vector.tensor_copy(ht[:, 1, :], hcts[1][:])
                nc.sync.dma_start(
                    hc1T.opt()[:, t * P:(t + 1) * P]
                    .rearrange("(a p) n -> p a n", p=P), ht[:])
                ep = ps.tile([P, 16], F32, tag="aux")
                nc.tensor.matmul(ep[:], lhsT=hcts[0][:], rhs=werC2[0][:],
                                 start=True, stop=False)
                nc.tensor.matmul(ep[:], lhsT=hcts[1][:], rhs=werC2[1][:],
                                 start=False, stop=True)
                nc.scalar.copy(erC2s[:, t * 16:(t + 1) * 16], ep[:])

            for t in range(CT):
                sem_tile(0, oDC1T.opt(), oCC1T.opt(), t, sem1_consume)
            # local tabCC2 shard from local hc1, then AllGather the table
            proj(dram_lhs(hc1T.opt().rearrange("(a p) n -> a p n", p=P)),
                 CT, [(wt['WCC2'], tabCC2loc.opt())], BT=CT)
            nc.gpsimd.collective_compute(
                "AllGather", AluOp.bypass,
                replica_groups=[list(range(nco))],
                ins=[tabCC2loc.opt()], outs=[tabCC2.opt()])

            # ---- L2 edge phases
            edge_phase('dc',
                       [tabDC2.opt()[0:HALF, :], tabDC2.opt()[HALF:NDP, :]],
                       lambda t: erC2s[:, t * 16:t * 16 + 8],
                       sched['dc'], make_epi(3, oDC2T.opt()))
            edge_phase('cc', [tabCC2.opt()[:, :]],
                       lambda t: erC2s[:, t * 16 + 8:t * 16 + 16],
                       sched['cc'], make_epi(4, oCC2T.opt()))

            # ---- sem2 + MLP head
            def sem2_consume(t, hcts):
                h1p = ps.tile([32, P], F32, tag="aux")
                nc.tensor.matmul(h1p[:], lhsT=dW1[0][:], rhs=hcts[0][:],
                                 start=True, stop=False)
                nc.tensor.matmul(h1p[:], lhsT=dW1[1][:], rhs=hcts[1][:],
                                 start=False, stop=True)
                h1 = sb.tile([32, P], BF16, tag="mlph1")
                nc.vector.scalar_tensor_tensor(
                    out=h1[:], in0=h1p[:], scalar=1.0,
                    in1=db1[:].to_broadcast([32, P]),
                    op0=AluOp.mult, op1=AluOp.add)
                nc.vector.scalar_tensor_tensor(
                    out=h1[:], in0=h1[:], scalar=0.01, in1=h1[:],
                    op0=AluOp.mult, op1=AluOp.max)
                h2p = ps.tile([16, P], F32, tag="aux")
                nc.tensor.matmul(h2p[:], lhsT=dW2[:], rhs=h1[:],
                                 start=True, stop=True)
                h2 = sb.tile([16, P], BF16, tag="mlph2")
                nc.vector.scalar_tensor_tensor(
                    out=h2[:], in0=h2p[:], scalar=1.0,
                    in1=db2[:].to_broadcast([16, P]),
                    op0=AluOp.mult, op1=AluOp.add)
                nc.vector.scalar_tensor_tensor(
                    out=h2[:], in0=h2[:], scalar=0.01, in1=h2[:],
                    op0=AluOp.mult, op1=AluOp.max)
                h3p = ps.tile([1, P], F32, tag="aux")
                nc.tensor.matmul(h3p[:], lhsT=dW3[:], rhs=h2[:],
                                 start=True, stop=True)
                h3 = sb.tile([1, P], F32, tag="mlph3")
                nc.vector.tensor_scalar(h3[:], h3p[:], db3[:], None,
                                        op0=AluOp.add)
                nc.sync.dma_start(out[0:1, t * P:(t + 1) * P], h3[:])

            for t in range(CT):
                sem_tile(1, oDC2T.opt(), oCC2T.opt(), t, sem2_consume)

    nc.compile()
    if legalize:
        legalize_waits(nc)
    return nc


# --------------------------------------------------------------------------
# entry point
# --------------------------------------------------------------------------

_CACHE = {}


def kernel(**inputs):
    cfg = make_cfg(inputs['feat_drug'].shape[0], inputs['feat_cell'].shape[0])
    sched, in_maps = host_prep(inputs, cfg)
    key = tuple(int(x) for s in sched.values() for x in s.flatten())
    if key not in _CACHE:
        _CACHE[key] = build_program(sched, cfg)
    nc = _CACHE[key]
    from concourse.bass_utils import run_bass_kernel_spmd
    res = run_bass_kernel_spmd(nc, in_maps, list(range(cfg['n_cores'])))
    pieces = [res.results[c]['out'][0] for c in range(cfg['n_cores'])]
    full = np.concatenate([p[:cfg['CBLK']] for p in pieces])[:cfg['Nc']]
    return full.reshape(-1, 1).astype(np.float32)
